# revision 1
# baseline (speedup 1.0000x reference)
"""Trainium2 Bass kernel for agent-attention (AAGA): 8-core data-parallel over batch.

Math (per batch b):
  qkv = x @ W_qkv + b_qkv ; q,k,v = split(qkv)
  ag  = agent @ W_agent + b_agent ; q_agent,k_agent = split(ag)
  attn1 = softmax(q_agent @ k^T * s)        # [K, N]
  va    = (attn1 @ v) @ W_fc1 + b_fc1       # [K, d]
  attn2 = softmax(q @ k_agent^T * s)        # [N, K]
  out   = (attn2 @ va) @ W_fc2 + b_fc2 + x  # [N, d]

Host-side algebraic folds (everything not involving x is an input):
  q_agent/k_agent computed on host; q,k,v never materialized on device.
  S1^T = x @ (W_k@q_agent^T): the b_k term is constant along the softmax axis
         and drops out (shift invariance).
  S2^T = (W_q@k_agent^T)^T @ x^T + c2 per-agent; exp(c2) is folded into the
         vaF rows (rec1 *= exp(c2)) and into bbig, keeping exp2 bias-free.
  va-chain: attn1 rows sum to 1, so all later biases fold:
       vaF = ((attn1@x) @ (W_v@W_fc1@W_fc2)) + ((b_v@W_fc1+b_fc1)@W_fc2 + b_fc2)
  A ones column rides x (-> stage-1 softmax sums) and vaF (-> s2 per token).
  Host epilogue: out = y_num / s2 + x   (exact fp32).

Device work per core (B=8 data-parallel, no collectives):
  S1^T matmuls -> exp -> avx accumulation (interleaved per xT chunk so it
  streams behind the input DMA), deferred S2^T -> exp, tiny vaF chain,
  y_ext = expS2^T.T @ vaF_ext, bf16 psum->sbuf copies, DMA out.
"""

import numpy as np
import ml_dtypes

B, N, D, K = 8, 4096, 256, 64
E = D + 1          # ones-column appended
P = 128
NT = N // P        # 32 token tiles
DS = D // P        # 2 contraction subtiles
W = 512            # free-dim chunk for S2^T
NC2 = N // W       # 8 chunks

_BF16 = ml_dtypes.bfloat16
_FP8 = ml_dtypes.float8_e4m3

_CACHE = {}


def _build_nc():
    import concourse.bass as bass
    import concourse.tile as tile
    from concourse import bacc, mybir

    f32 = mybir.dt.float32
    bf16 = mybir.dt.bfloat16
    fp8 = mybir.dt.float8e4
    Exp = mybir.ActivationFunctionType.Exp
    DR = mybir.MatmulPerfMode.DoubleRow
    Copy = mybir.ActivationFunctionType.Copy
    ts = bass.ts

    nc = bacc.Bacc("TRN2", target_bir_lowering=False, debug=False)

    x_d = nc.declare_dram_parameter("x", [N, E], fp8, isOutput=False)
    xT_d = nc.declare_dram_parameter("xT", [D, N], fp8, isOutput=False)
    WC = E
    FC = E + K + 1
    wcombo_d = nc.declare_dram_parameter("wcombo", [D, WC], bf16, isOutput=False)
    wkq8_d = nc.declare_dram_parameter("wkq8", [D, 2 * K], fp8, isOutput=False)
    fcombo_d = nc.declare_dram_parameter("fcombo", [K, FC], f32, isOutput=False)
    ye_d = nc.declare_dram_parameter("ye", [N, E], fp8, isOutput=True)

    with tile.TileContext(nc) as tc:
        with (
            tc.tile_pool(name="sb", bufs=1) as sb,
            tc.tile_pool(name="yout", bufs=6) as yout,
            tc.tile_pool(name="ps_a", bufs=4, space="PSUM") as ps_a,
            tc.tile_pool(name="ps_b", bufs=2, space="PSUM") as ps_b,
            tc.tile_pool(name="ps_c", bufs=2, space="PSUM") as ps_c,
        ):

            # ---------------- loads ----------------
            # all small constants ride two wide DMAs (big contiguous runs);
            # narrow per-tensor DMAs are descriptor-bound and stall the head
            wcombo = sb.tile([P, DS, WC], bf16)
            nc.sync.dma_start(
                out=wcombo, in_=wcombo_d.rearrange("(s p) k -> p s k", p=P)
            )
            wbig = wcombo[:, :, 0:WC]
            wkq8 = sb.tile([P, DS, 2 * K], fp8)
            nc.sync.dma_start(
                out=wkq8, in_=wkq8_d.rearrange("(s p) k -> p s k", p=P)
            )
            wk = wkq8[:, :, 0:K]
            wq = wkq8[:, :, K : 2 * K]
            fcombo0 = sb.tile([K, FC], f32)
            nc.sync.dma_start(out=fcombo0, in_=fcombo_d[:, :])
            fcombo = sb.tile([K, FC], f32)
            nc.vector.tensor_copy(fcombo, fcombo0)   # stage once via DVE
            bbig = fcombo[:, 0:E]
            ec2 = fcombo[:, E : E + 1]
            ident = fcombo[:, E + 1 : E + 1 + K]

            xT = sb.tile([P, DS, N], fp8)
            xT_r = xT_d.rearrange("(s p) n -> p s n", p=P)
            xe = sb.tile([P, NT, E], fp8)
            xe_r = x_d.rearrange("(t p) c -> p t c", p=P)
            for c in range(8):
                nc.sync.dma_start(
                    out=xT[:, :, ts(c, N // 8)], in_=xT_r[:, :, ts(c, N // 8)]
                )
                if c < 4:       # xe chunk b feeds avx tiles 8b..8b+7
                    nc.sync.dma_start(
                        out=xe[:, ts(c, NT // 4), :], in_=xe_r[:, ts(c, NT // 4), :]
                    )

            expS1 = sb.tile([P, NT, K], fp8)    # token-major exp(S1), /16-shifted
            sh1 = sb.tile([P, 1], f32)
            nc.vector.memset(sh1, -2.772588722239781)   # -ln(16): keeps exp < 240 (fp8 max)
            expS2 = sb.tile([K, N], bf16)       # agent-major exp(S2)

            # ---- per bank-group: S1T logits -> exp -> avx accumulation ----
            # group b covers token tiles 8b..8b+7 == xT chunks 2b,2b+1 == xe
            # chunk b; avx completes right behind the input DMA stream. S2T is
            # deferred: it only feeds the y matmuls, which wait on vaF anyway.
            avx_ps = ps_c.tile([K, E], f32, tag="psc")
            GRP = 4
            for b in range(NT // GRP):
                ps = ps_a.tile([P, GRP, K], f32, tag="psa")
                for j in range(GRP):
                    t = b * GRP + j
                    # DoubleRow: 2 fp8 weights/cell -> full 256-contraction in one mm
                    nc.tensor.matmul(
                        ps[:, j, :], xT[:, :, ts(t, P)], wk,
                        start=True, stop=True, perf_mode=DR,
                    )
                # b_k drops out of softmax; scale = 1/sqrt(D)
                nc.scalar.activation(
                    expS1[:, ts(b, GRP), :], ps, Exp,
                    scale=float(D ** -0.5), bias=sh1,
                )
                for j in range(GRP // 2):
                    u = b * (GRP // 2) + j
                    nc.tensor.matmul(
                        avx_ps, expS1[:, 2 * u : 2 * u + 2, :],
                        xe[:, 2 * u : 2 * u + 2, :],
                        start=(u == 0), stop=(u == NT // 2 - 1), perf_mode=DR,
                    )

            # ---- stage 2 logits (agent-major), overlaps the vaF chain ----
            for c in range(NC2):
                ps2 = ps_b.tile([P, W], f32, tag="psb")
                nc.tensor.matmul(
                    ps2[:K, :], wq, xT[:, :, ts(c, W)],
                    start=True, stop=True, perf_mode=DR,
                )
                nc.scalar.activation(
                    expS2[:, ts(c, W)], ps2[:K, :], Exp,
                    scale=float(D ** -0.5), bias=sh1[:K, :],
                )

            rec1 = sb.tile([K, 1], f32)
            nc.vector.reciprocal(rec1, avx_ps[:, D:E])
            nc.vector.tensor_tensor(rec1, rec1, ec2, mybir.AluOpType.mult)

            # ---- vaF = (avx*ec2/s1 @ Wbig_ext) + bbig_ext : [K, E] ----
            avx_s = sb.tile([K, D], f32)
            nc.vector.tensor_scalar_mul(avx_s, avx_ps[:, :D], rec1)
            avxT = sb.tile([P, DS, K], bf16)
            for s in range(DS):
                tp = ps_c.tile([P, K], f32, tag="psc")
                nc.tensor.transpose(tp, avx_s[:, ts(s, P)], ident)
                nc.vector.tensor_copy(avxT[:, s, :], tp)
            vf_ps = ps_c.tile([K, E], f32, tag="psc")
            for s in range(DS):
                nc.tensor.matmul(
                    vf_ps, avxT[:, s, :], wbig[:, s, :],
                    start=(s == 0), stop=(s == DS - 1),
                )
            vaF = sb.tile([K, E], bf16)
            nc.vector.tensor_tensor(vaF, vf_ps, bbig, mybir.AluOpType.add)

            # ---- y_ext[n, :] = sum_kk expS2[kk,n] * vaF_ext[kk, :] ----
            # col D of vaF_ext is exp(c2), so col D of y_ext = s2 (softmax
            # denominator). Casts alternate DVE/ACT; two tiles share one DMA.
            for u in range(NT // 2):
                y_sb = yout.tile([P, 2, E], fp8, tag="ysb")
                for j in range(2):
                    t = 2 * u + j
                    yp = ps_a.tile([P, E], f32, tag="psa")
                    nc.tensor.matmul(
                        yp, expS2[:, ts(t, P)], vaF, start=True, stop=True
                    )
                    if t % 2 == 0:
                        nc.vector.tensor_copy(y_sb[:, j, :], yp)
                    else:
                        nc.scalar.activation(y_sb[:, j, :], yp, Copy)
                nc.sync.dma_start(
                    out=ye_d.rearrange("(u p) c -> p u c", p=P)[:, ts(u, 2), :],
                    in_=y_sb,
                )

    nc.compile()
    return nc


def _get_nc():
    if "nc" not in _CACHE:
        _CACHE["nc"] = _build_nc()
    return _CACHE["nc"]


def _prepare_in_maps(agent, x, W_qkv, b_qkv, W_agent, b_agent, W_fc1, b_fc1, W_fc2, b_fc2):
    # ---- host folds (float64 for stability, cast down at the end) ----
    agent64 = np.asarray(agent, np.float64)
    Wqkv64 = np.asarray(W_qkv, np.float64)
    bqkv64 = np.asarray(b_qkv, np.float64)
    Wag64 = np.asarray(W_agent, np.float64)
    bag64 = np.asarray(b_agent, np.float64)
    Wf1 = np.asarray(W_fc1, np.float64)
    bf1 = np.asarray(b_fc1, np.float64)
    Wf2 = np.asarray(W_fc2, np.float64)
    bf2 = np.asarray(b_fc2, np.float64)

    ag = agent64 @ Wag64 + bag64
    q_agent, k_agent = ag[:, :D], ag[:, D:]
    W_q, W_k, W_v = Wqkv64[:, :D], Wqkv64[:, D : 2 * D], Wqkv64[:, 2 * D :]
    b_q, b_v = bqkv64[:D], bqkv64[2 * D :]

    wk_f = W_k @ q_agent.T                      # [D, K]
    wq_f = W_q @ k_agent.T                      # [D, K]
    c2_f = (D ** -0.5) * (k_agent @ b_q)        # [K]
    ec2_f = np.exp(c2_f)                        # [K]
    Wbig = W_v @ Wf1 @ Wf2                      # [D, D]
    bbig = (b_v @ Wf1 + bf1) @ Wf2 + bf2        # [D]

    wcombo = np.zeros((D, E), np.float32)
    wcombo[:, :D] = Wbig
    wcombo_b = wcombo.astype(_BF16)
    wkq8 = np.concatenate([wk_f, wq_f], axis=1).astype(_FP8)
    bbig_e1 = np.zeros((1, E), np.float64)
    bbig_e1[0, :D] = bbig
    bbig_e1[0, D] = 1.0        # -> vaF_ext col D = exp(c2); y_ext col D = s2
    fcombo = np.zeros((K, E + K + 1), np.float32)
    fcombo[:, :E] = ec2_f[:, None] * bbig_e1
    fcombo[:, E] = ec2_f
    fcombo[:, E + 1 :] = np.eye(K)

    x32 = np.asarray(x, np.float32)
    xb = np.ones((B, N, E), _FP8)
    xb[:, :, :D] = x32.astype(_FP8)                               # [B, N, D+1]
    xTb = np.ascontiguousarray(x32.transpose(0, 2, 1)).astype(_FP8)   # [B, D, N]

    in_maps = [
        {
            "x": xb[i],
            "xT": xTb[i],
            "wcombo": wcombo_b,
            "wkq8": wkq8,
            "fcombo": fcombo,
        }
        for i in range(B)
    ]

    return in_maps, x32


def kernel(**inputs):
    from concourse.bass_utils import run_bass_kernel_spmd

    in_maps, x32 = _prepare_in_maps(**inputs)
    nc = _get_nc()
    res_obj = run_bass_kernel_spmd(nc, in_maps, core_ids=list(range(B)))
    _CACHE["last_results"] = res_obj
    res = res_obj.results

    ye = np.stack([np.asarray(res[i]["ye"]) for i in range(B)]).astype(np.float32)
    out = ye[:, :, :D] / ye[:, :, D:E] + x32
    return out.astype(np.float32)



# revision 7
# speedup vs baseline: 1.1032x; 1.1032x over previous
"""Trainium2 Bass kernel for agent-attention (AAGA): 8-core data-parallel over batch.

Math (per batch b):
  qkv = x @ W_qkv + b_qkv ; q,k,v = split(qkv)
  ag  = agent @ W_agent + b_agent ; q_agent,k_agent = split(ag)
  attn1 = softmax(q_agent @ k^T * s)        # [K, N]
  va    = (attn1 @ v) @ W_fc1 + b_fc1       # [K, d]
  attn2 = softmax(q @ k_agent^T * s)        # [N, K]
  out   = (attn2 @ va) @ W_fc2 + b_fc2 + x  # [N, d]

Host-side algebraic folds (everything not involving x is an input):
  q_agent/k_agent computed on host; q,k,v never materialized on device.
  S1^T = x @ (W_k@q_agent^T): b_k drops out of the softmax (shift invariance).
  va-chain: attn1 rows sum to 1, so all later biases fold into a single
       constant row bbig = (b_v@W_fc1+b_fc1)@W_fc2 + b_fc2 ADDED ON HOST.
  Device vaF[k,:] = [ (ec2/s1)[k] * (expS1^T x)[k,:] @ Wbig | ec2[k] ],
  with Wbig = W_v@W_fc1@W_fc2. Then y = expS2^T @ vaF gives
  y[:, :D] = attn2-numerator combination and y[:, D] = s2 (denominator).
  Host epilogue: out = y[:, :D]/y[:, D] + bbig + x  (exact fp32).

DMA/engine regime (cost model): each DMACopy costs ~625ns on a single
serialized HWDGE queue (SP/Act) or ~1us on the Pool engine via SWDGE;
transfers serialize on DMA_ENGINES at 360GB/s with a 2x penalty for
runs <512B. GPSIMD cannot touch PSUM, so all PSUM->SBUF casts go on
DVE+Act, batched 3-4 token-tiles per instruction via multi-bank PSUM
tiles, alternating two PSUM pools so matmul and copy pipeline.
"""

import numpy as np
import ml_dtypes

B, N, D, K = 8, 4096, 256, 64
E = D + 1          # ones-column appended
P = 128
NT = N // P        # 32 token tiles
DS = D // P        # 2 contraction subtiles
W = 512            # free-dim chunk for S2^T
NC2 = N // W       # 8 chunks

# input streaming chunks (in token tiles); last chunks small to cut tail lag
XCHUNKS = [8, 8, 8, 6, 2]
# y-phase groups (in token tiles): alternate 4-tile/3-tile PSUM pools
YGROUPS = [4, 3, 4, 3, 4, 3, 4, 3, 4]

_BF16 = ml_dtypes.bfloat16
_FP8 = ml_dtypes.float8_e4m3

_CACHE = {}


def _build_nc():
    import concourse.bass as bass
    import concourse.tile as tile
    from concourse import bacc, mybir

    f32 = mybir.dt.float32
    bf16 = mybir.dt.bfloat16
    fp8 = mybir.dt.float8e4
    Exp = mybir.ActivationFunctionType.Exp
    DR = mybir.MatmulPerfMode.DoubleRow
    Copy = mybir.ActivationFunctionType.Copy
    ts = bass.ts

    nc = bacc.Bacc("TRN2", target_bir_lowering=False, debug=False)

    xT_d = nc.declare_dram_parameter("xT", [P, DS, N], fp8, isOutput=False)
    xe_d = nc.declare_dram_parameter("xe", [P, NT, E], fp8, isOutput=False)
    wkq8_d = nc.declare_dram_parameter("wkq8", [P, DS, 2 * K], fp8, isOutput=False)
    wcombo_d = nc.declare_dram_parameter("wcombo", [P, DS, D], bf16, isOutput=False)
    FC = 1 + K         # [ec2 | I64]
    fcombo_d = nc.declare_dram_parameter("fcombo", [K, FC], bf16, isOutput=False)
    ye_d = nc.declare_dram_parameter("ye", [P, NT, E], fp8, isOutput=True)

    with tile.TileContext(nc) as tc:
        with (
            tc.tile_pool(name="sb", bufs=1) as sb,
            tc.tile_pool(name="yout", bufs=2) as yout,
            tc.tile_pool(name="pA", bufs=1, space="PSUM") as pA,   # 4 banks
            tc.tile_pool(name="pB", bufs=1, space="PSUM") as pB,   # 3 banks
            tc.tile_pool(name="pX", bufs=1, space="PSUM") as pX,   # 1 bank (avx)
        ):
            # ---------------- input DMAs ----------------
            wkq8 = sb.tile([P, DS, 2 * K], fp8)
            nc.sync.dma_start(out=wkq8, in_=wkq8_d[:, :, :])
            wk = wkq8[:, :, 0:K]
            wq = wkq8[:, :, K : 2 * K]

            xT = sb.tile([P, DS, N], fp8)
            xe = sb.tile([P, NT, E], fp8)
            t0 = 0
            for ci, ct in enumerate(XCHUNKS):
                nc.sync.dma_start(
                    out=xT[:, :, P * t0 : P * (t0 + ct)],
                    in_=xT_d[:, :, P * t0 : P * (t0 + ct)],
                )
                nc.gpsimd.dma_start(
                    out=xe[:, t0 : t0 + ct, :], in_=xe_d[:, t0 : t0 + ct, :]
                )
                if ci == 2:
                    wcombo = sb.tile([P, DS, D], bf16)
                    nc.sync.dma_start(out=wcombo, in_=wcombo_d[:, :, :])
                    fcombo = sb.tile([K, FC], bf16)
                    nc.sync.dma_start(out=fcombo, in_=fcombo_d[:, :])
                t0 += ct
            ec2 = fcombo[:, 0:1]
            ident = fcombo[:, 1 : 1 + K]            # I64 bf16

            expS1 = sb.tile([P, NT, K], fp8)    # token-major exp(S1), /16-shifted
            sh1 = sb.tile([P, 1], f32)
            nc.vector.memset(sh1, -2.772588722239781)   # -ln(16): keeps exp < 240 (fp8 max)
            expS2 = sb.tile([K, NC2, W], bf16)          # agent-major exp(S2)

            # vaF holder; constant ec2 column written early
            vaF = sb.tile([K, E], bf16)
            nc.vector.tensor_copy(vaF[:, D:E], ec2)

            # ---- phase 1: S1 slabs + paired S2 chunks, 3 PSUM tiles / chunk ----
            avx_ps = pX.tile([K, E], f32, tag="pX")
            SL = 4                 # token tiles per S1 slab
            pools = [pA, pB]

            def s1_slab(b, pool):
                ps = pool.tile([P, SL, K], f32, tag=pool.name)
                for j in range(SL):
                    t = b * SL + j
                    # DoubleRow: 2 fp8 weights/cell -> full 256-contraction in one mm
                    nc.tensor.matmul(
                        ps[:, j, :], xT[:, :, ts(t, P)], wk,
                        start=True, stop=True, perf_mode=DR,
                    )
                nc.scalar.activation(
                    expS1[:, ts(b, SL), :], ps, Exp,
                    scale=float(D ** -0.5), bias=sh1,
                )
                for j in range(SL // 2):
                    u = b * (SL // 2) + j
                    nc.tensor.matmul(
                        avx_ps, expS1[:, 2 * u : 2 * u + 2, :],
                        xe[:, 2 * u : 2 * u + 2, :],
                        start=(u == 0), stop=(u == NT // 2 - 1), perf_mode=DR,
                    )

            def s2_pair(h, pool):
                # two 512-token chunks side by side in two PSUM banks ->
                # one exp instruction of free size 1024
                p2 = pool.tile([K, 2, W], f32, tag=pool.name)
                for g in range(2):
                    c = 2 * h + g
                    nc.tensor.matmul(
                        p2[:, g, :], wq, xT[:, :, ts(c, W)],
                        start=True, stop=True, perf_mode=DR,
                    )
                nc.scalar.activation(
                    expS2[:, 2 * h : 2 * h + 2, :], p2, Exp,
                    scale=float(D ** -0.5), bias=sh1[0:K, :],
                )

            pi = 0
            for b in range(NT // SL):          # 8 slabs of 4 tiles
                s1_slab(b, pools[pi % 2]); pi += 1
                if b % 2 == 1:
                    s2_pair(b // 2, pools[pi % 2]); pi += 1

            # ---- vaF[:, :D] = (avx*ec2/s1 @ Wbig), duplicated on 128 parts ----
            rec1 = sb.tile([K, 1], f32)
            nc.vector.reciprocal(rec1, avx_ps[:, D:E])
            nc.vector.tensor_tensor(rec1, rec1, ec2[0:K, :], mybir.AluOpType.mult)
            avx_s = sb.tile([K, D], bf16)
            nc.vector.tensor_scalar_mul(avx_s, avx_ps[:, :D], rec1)
            avxT = sb.tile([P, DS, K], bf16)
            tp = pB.tile([P, DS, K], bf16, tag="pB")
            for s in range(DS):
                nc.tensor.transpose(tp[:, s, :], avx_s[:, ts(s, P)], ident)
            nc.vector.tensor_copy(avxT, tp)        # bf16: DVE 2x mode
            vf_ps = pB.tile([K, D], f32, tag="pB")
            for s in range(DS):
                nc.tensor.matmul(
                    vf_ps, avxT[:, s, :], wcombo[:, s, :],
                    start=(s == 0), stop=(s == DS - 1),
                )
            nc.scalar.activation(vaF[:, 0:D], vf_ps, Copy)

            # ---- y_ext[n, :] = sum_k expS2[k,n] * vaF_ext[k, :] ----
            # col D of vaF_ext is ec2, so col D of y_ext = s2. Groups of 4/3
            # tiles alternate PSUM pools; one batched cast per group rotates
            # DVE/Act; one DMA out per 7-8 tiles.
            ng = len(YGROUPS)
            gstart = [sum(YGROUPS[:i]) for i in range(ng)]
            y_sb = None
            ysb0 = 0
            cp = 0
            for gi, gsz in enumerate(YGROUPS):
                pool = pools[gi % 2]
                g0 = gstart[gi]
                if gi % 2 == 0:
                    ysb0 = g0
                    ysz = gsz if gi == ng - 1 else gsz + YGROUPS[gi + 1]
                    y_sb = yout.tile([P, ysz, E], fp8, tag="ysb")
                yp = pool.tile([P, gsz, W], f32, tag=pool.name)
                for j in range(gsz):
                    t = g0 + j
                    ch = t // 4
                    nc.tensor.matmul(
                        yp[:, j, 0:E],
                        expS2[:, ch, ts(t % 4, P)],
                        vaF, start=True, stop=True,
                    )
                dst = y_sb[:, g0 - ysb0 : g0 - ysb0 + gsz, :]
                if cp % 2 == 0:
                    nc.vector.tensor_copy(dst, yp[:, :, 0:E])
                else:
                    nc.scalar.activation(dst, yp[:, :, 0:E], Copy)
                cp += 1
                if gi % 2 == 1 or gi == ng - 1:
                    qn = g0 + gsz - ysb0
                    nc.sync.dma_start(
                        out=ye_d[:, ysb0 : ysb0 + qn, :], in_=y_sb[:, 0:qn, :]
                    )

    nc.compile()
    return nc


def _get_nc():
    if "nc" not in _CACHE:
        _CACHE["nc"] = _build_nc()
    return _CACHE["nc"]


def _prepare_in_maps(agent, x, W_qkv, b_qkv, W_agent, b_agent, W_fc1, b_fc1, W_fc2, b_fc2):
    # ---- host folds (float64 for stability, cast down at the end) ----
    agent64 = np.asarray(agent, np.float64)
    Wqkv64 = np.asarray(W_qkv, np.float64)
    bqkv64 = np.asarray(b_qkv, np.float64)
    Wag64 = np.asarray(W_agent, np.float64)
    bag64 = np.asarray(b_agent, np.float64)
    Wf1 = np.asarray(W_fc1, np.float64)
    bf1 = np.asarray(b_fc1, np.float64)
    Wf2 = np.asarray(W_fc2, np.float64)
    bf2 = np.asarray(b_fc2, np.float64)

    ag = agent64 @ Wag64 + bag64
    q_agent, k_agent = ag[:, :D], ag[:, D:]
    W_q, W_k, W_v = Wqkv64[:, :D], Wqkv64[:, D : 2 * D], Wqkv64[:, 2 * D :]
    b_q, b_v = bqkv64[:D], bqkv64[2 * D :]

    wk_f = W_k @ q_agent.T                      # [D, K]
    wq_f = W_q @ k_agent.T                      # [D, K]
    c2_f = (D ** -0.5) * (k_agent @ b_q)        # [K]
    ec2_f = np.exp(c2_f)                        # [K]
    Wbig = W_v @ Wf1 @ Wf2                      # [D, D]
    bbig = (b_v @ Wf1 + bf1) @ Wf2 + bf2        # [D], added on host

    # [D, D] -> [P, DS, D] with d = s*128 + p
    wcombo_b = np.ascontiguousarray(
        Wbig.reshape(DS, P, D).transpose(1, 0, 2)
    ).astype(_BF16)
    wkq8 = np.concatenate([wk_f, wq_f], axis=1).reshape(DS, P, 2 * K)
    wkq8 = np.ascontiguousarray(wkq8.transpose(1, 0, 2)).astype(_FP8)
    fcombo = np.zeros((K, 1 + K), np.float32)
    fcombo[:, 0] = ec2_f
    fcombo[:, 1 :] = np.eye(K)
    fcombo = np.ascontiguousarray(fcombo).astype(_BF16)

    x32 = np.asarray(x, np.float32)
    # xe pack: [B, N, E] -> [B, P, NT, E], token = t*128 + p
    xb = np.ones((B, N, E), _FP8)
    xb[:, :, :D] = x32.astype(_FP8)
    xeb = np.ascontiguousarray(xb.reshape(B, NT, P, E).transpose(0, 2, 1, 3))
    # xT pack: [B, D, N] -> [B, P, DS, N], d = s*128 + p
    xTb = x32.transpose(0, 2, 1).reshape(B, DS, P, N)
    xTb = np.ascontiguousarray(xTb.transpose(0, 2, 1, 3)).astype(_FP8)

    in_maps = [
        {
            "xT": xTb[i],
            "xe": xeb[i],
            "wkq8": wkq8,
            "wcombo": wcombo_b,
            "fcombo": fcombo,
        }
        for i in range(B)
    ]

    return in_maps, x32, bbig.astype(np.float32)


def kernel(**inputs):
    from concourse.bass_utils import run_bass_kernel_spmd

    in_maps, x32, bbig = _prepare_in_maps(**inputs)
    nc = _get_nc()
    res_obj = run_bass_kernel_spmd(nc, in_maps, core_ids=list(range(B)))
    _CACHE["last_results"] = res_obj
    res = res_obj.results

    # ye [P, NT, E] -> [N, E] with token = t*128 + p
    ye = np.stack(
        [np.asarray(res[i]["ye"]).transpose(1, 0, 2).reshape(N, E) for i in range(B)]
    ).astype(np.float32)
    out = ye[:, :, :D] / ye[:, :, D:E] + bbig[None, None, :] + x32
    return out.astype(np.float32)


# revision 8
# speedup vs baseline: 1.1725x; 1.0629x over previous
"""Trainium2 Bass kernel for agent-attention (AAGA): 8-core data-parallel over batch.

Math (per batch b):
  qkv = x @ W_qkv + b_qkv ; q,k,v = split(qkv)
  ag  = agent @ W_agent + b_agent ; q_agent,k_agent = split(ag)
  attn1 = softmax(q_agent @ k^T * s)        # [K, N]
  va    = (attn1 @ v) @ W_fc1 + b_fc1       # [K, d]
  attn2 = softmax(q @ k_agent^T * s)        # [N, K]
  out   = (attn2 @ va) @ W_fc2 + b_fc2 + x  # [N, d]

Host-side algebraic folds (everything not involving x is an input):
  q_agent/k_agent computed on host; q,k,v never materialized on device.
  S1^T = x @ (W_k@q_agent^T): b_k drops out of the softmax (shift invariance).
  va-chain: attn1 rows sum to 1, so all later biases fold into a single
       constant row bbig = (b_v@W_fc1+b_fc1)@W_fc2 + b_fc2 ADDED ON HOST.
  Device vaF[k,:] = [ (ec2/s1)[k] * (expS1^T x)[k,:] @ Wbig | ec2[k] ],
  with Wbig = W_v@W_fc1@W_fc2. Then y = expS2^T @ vaF gives
  y[:, :D] = attn2-numerator combination and y[:, D] = s2 (denominator).
  Host epilogue: out = y[:, :D]/y[:, D] + bbig + x  (exact fp32).

DMA/engine regime (cost model): each DMACopy costs ~625ns on a single
serialized HWDGE queue (SP/Act) or ~1us on the Pool engine via SWDGE;
transfers serialize on DMA_ENGINES at 360GB/s with a 2x penalty for
runs <512B. GPSIMD cannot touch PSUM, so all PSUM->SBUF casts go on
DVE+Act, batched 3-4 token-tiles per instruction via multi-bank PSUM
tiles, alternating two PSUM pools so matmul and copy pipeline.
"""

import numpy as np
import ml_dtypes

B, N, D, K = 8, 4096, 256, 64
E = D + 1          # ones-column appended
P = 128
NT = N // P        # 32 token tiles
DS = D // P        # 2 contraction subtiles
W = 512            # free-dim chunk for S2^T
NC2 = N // W       # 8 chunks

# input streaming chunks (in token tiles); last chunks small to cut tail lag
XCHUNKS = [8, 8, 8, 6, 2]
# S1 slabs (in token tiles); small last slabs shorten the avx tail
SLABS = [8, 8, 8, 4, 2, 2]
# y-phase groups (in token tiles); small last groups shorten the DMA tail
YGROUPS = [4, 4, 4, 4, 4, 4, 4, 2, 2]

_BF16 = ml_dtypes.bfloat16
_FP8 = ml_dtypes.float8_e4m3

_CACHE = {}


def _build_nc():
    import concourse.bass as bass
    import concourse.tile as tile
    from concourse import bacc, mybir

    f32 = mybir.dt.float32
    bf16 = mybir.dt.bfloat16
    fp8 = mybir.dt.float8e4
    Exp = mybir.ActivationFunctionType.Exp
    DR = mybir.MatmulPerfMode.DoubleRow
    Copy = mybir.ActivationFunctionType.Copy
    ts = bass.ts

    nc = bacc.Bacc("TRN2", target_bir_lowering=False, debug=False)

    xT_d = nc.declare_dram_parameter("xT", [P, DS, N], fp8, isOutput=False)
    xe_d = nc.declare_dram_parameter("xe", [P, NT, E], fp8, isOutput=False)
    wkq8_d = nc.declare_dram_parameter("wkq8", [P, DS, 2 * K], fp8, isOutput=False)
    wcombo_d = nc.declare_dram_parameter("wcombo", [P, DS, D], bf16, isOutput=False)
    FC = 1 + K         # [ec2 | I64]
    fcombo_d = nc.declare_dram_parameter("fcombo", [K, FC], bf16, isOutput=False)
    ye_d = nc.declare_dram_parameter("ye", [P, NT, E], fp8, isOutput=True)

    with tile.TileContext(nc) as tc:
        with (
            tc.tile_pool(name="sb", bufs=1) as sb,
            tc.tile_pool(name="yout", bufs=4) as yout,
        ):
            # ---------------- input DMAs ----------------
            wkq8 = sb.tile([P, DS, 2 * K], fp8)
            nc.sync.dma_start(out=wkq8, in_=wkq8_d[:, :, :])
            wk = wkq8[:, :, 0:K]
            wq = wkq8[:, :, K : 2 * K]

            xT = sb.tile([P, DS, N], fp8)
            xe = sb.tile([P, NT, E], fp8)
            t0 = 0
            for ci, ct in enumerate(XCHUNKS):
                nc.sync.dma_start(
                    out=xT[:, :, P * t0 : P * (t0 + ct)],
                    in_=xT_d[:, :, P * t0 : P * (t0 + ct)],
                )
                nc.gpsimd.dma_start(
                    out=xe[:, t0 : t0 + ct, :], in_=xe_d[:, t0 : t0 + ct, :]
                )
                if ci == 2:
                    wcombo = sb.tile([P, DS, D], bf16)
                    nc.sync.dma_start(out=wcombo, in_=wcombo_d[:, :, :])
                    fcombo = sb.tile([K, FC], bf16)
                    nc.sync.dma_start(out=fcombo, in_=fcombo_d[:, :])
                t0 += ct
            ec2 = fcombo[:, 0:1]
            ident = fcombo[:, 1 : 1 + K]            # I64 bf16

            expS1 = sb.tile([P, NT, K], fp8)    # token-major exp(S1), /16-shifted
            sh1 = sb.tile([P, 1], f32)
            nc.vector.memset(sh1, -2.772588722239781)   # -ln(16): keeps exp < 240 (fp8 max)
            expS2 = sb.tile([K, NC2, W], bf16)          # agent-major exp(S2)

            # vaF holder; constant ec2 column written early
            vaF = sb.tile([K, E], bf16)
            nc.vector.tensor_copy(vaF[:, D:E], ec2)

            # ---- phase 1: S1 slabs + free-axis-paired S2 chunks ----
            # deep PSUM rings so matmul(n+1) never waits exp(n); scoped so the
            # y phase can reuse all 8 banks afterwards
            with (
                tc.tile_pool(name="s1p", bufs=3, space="PSUM") as s1p,  # 3 banks
                tc.tile_pool(name="s2p", bufs=2, space="PSUM") as s2p,  # 4 banks
                tc.tile_pool(name="pX", bufs=1, space="PSUM") as pX,    # 1 bank
            ):
                avx_ps = pX.tile([K, E], f32, tag="pX")
                nslab = len(SLABS)
                sstart = [sum(SLABS[:i]) for i in range(nslab)]

                def s1_slab(b):
                    t0, sl = sstart[b], SLABS[b]
                    ps = s1p.tile([P, sl, K], f32, tag="s1p")
                    for j in range(sl):
                        t = t0 + j
                        # DoubleRow: 2 fp8 weights/cell -> 256-contraction in one mm
                        nc.tensor.matmul(
                            ps[:, j, :], xT[:, :, ts(t, P)], wk,
                            start=True, stop=True, perf_mode=DR,
                        )
                    nc.scalar.activation(
                        expS1[:, t0 : t0 + sl, :], ps, Exp,
                        scale=float(D ** -0.5), bias=sh1,
                    )
                    for j in range(sl // 2):
                        u = t0 // 2 + j
                        nc.tensor.matmul(
                            avx_ps, expS1[:, 2 * u : 2 * u + 2, :],
                            xe[:, 2 * u : 2 * u + 2, :],
                            start=(u == 0), stop=(u == NT // 2 - 1), perf_mode=DR,
                        )

                def s2_pair(h):
                    # two 512-token chunks in two PSUM banks -> one exp, free 1024
                    p2 = s2p.tile([K, 2, W], f32, tag="s2p")
                    for g in range(2):
                        c = 2 * h + g
                        nc.tensor.matmul(
                            p2[:, g, :], wq, xT[:, :, ts(c, W)],
                            start=True, stop=True, perf_mode=DR,
                        )
                    nc.scalar.activation(
                        expS2[:, 2 * h : 2 * h + 2, :], p2, Exp,
                        scale=float(D ** -0.5), bias=sh1[0:K, :],
                    )

                s2done = 0
                for b in range(nslab):
                    s1_slab(b)
                    # issue an s2 pair once its 8 token tiles of xT are covered
                    while s2done < NC2 // 2 and sstart[b] + SLABS[b] >= 8 * (s2done + 1):
                        s2_pair(s2done); s2done += 1

                # ---- vaF[:, :D] = (avx*ec2/s1 @ Wbig) ----
                rec1 = sb.tile([K, 1], f32)
                nc.vector.reciprocal(rec1, avx_ps[:, D:E])
                nc.vector.tensor_tensor(rec1, rec1, ec2, mybir.AluOpType.mult)
                avx_s = sb.tile([K, D], bf16)
                nc.vector.tensor_scalar_mul(avx_s, avx_ps[:, :D], rec1)
                avxT = sb.tile([P, DS, K], bf16)
                tp = s1p.tile([P, DS, K], bf16, tag="s1p")
                for s in range(DS):
                    nc.tensor.transpose(tp[:, s, :], avx_s[:, ts(s, P)], ident)
                nc.vector.tensor_copy(avxT[:, 0, :], tp[:, 0, :])
                nc.scalar.activation(avxT[:, 1, :], tp[:, 1, :], Copy)
                vf_ps = s1p.tile([K, D], f32, tag="s1p")
                for s in range(DS):
                    nc.tensor.matmul(
                        vf_ps, avxT[:, s, :], wcombo[:, s, :],
                        start=(s == 0), stop=(s == DS - 1),
                    )
                nc.vector.tensor_copy(vaF[:, 0 : D // 2], vf_ps[:, 0 : D // 2])
                nc.scalar.activation(vaF[:, D // 2 : D], vf_ps[:, D // 2 : D], Copy)

            # ---- y_ext[n, :] = sum_k expS2[k,n] * vaF_ext[k, :] ----
            # col D of vaF_ext is ec2, so col D of y_ext = s2. 4-tile groups in
            # a bufs=2 pool (4 banks each) so matmul and cast pipeline; casts
            # alternate DVE/Act; one DMA out per group.
            with tc.tile_pool(name="ypool", bufs=2, space="PSUM") as ypool:
                g0 = 0
                for gi, gsz in enumerate(YGROUPS):
                    yp = ypool.tile([P, 4, W], f32, tag="ypool")
                    y_sb = yout.tile([P, gsz, E], fp8, tag="ysb")
                    for j in range(gsz):
                        t = g0 + j
                        nc.tensor.matmul(
                            yp[:, j, 0:E],
                            expS2[:, t // 4, ts(t % 4, P)],
                            vaF, start=True, stop=True,
                        )
                    if gi % 2 == 0:
                        nc.vector.tensor_copy(y_sb, yp[:, 0:gsz, 0:E])
                    else:
                        nc.scalar.activation(y_sb, yp[:, 0:gsz, 0:E], Copy)
                    nc.sync.dma_start(
                        out=ye_d[:, g0 : g0 + gsz, :], in_=y_sb
                    )
                    g0 += gsz

    nc.compile()
    return nc


def _get_nc():
    if "nc" not in _CACHE:
        _CACHE["nc"] = _build_nc()
    return _CACHE["nc"]


def _prepare_in_maps(agent, x, W_qkv, b_qkv, W_agent, b_agent, W_fc1, b_fc1, W_fc2, b_fc2):
    # ---- host folds (float64 for stability, cast down at the end) ----
    agent64 = np.asarray(agent, np.float64)
    Wqkv64 = np.asarray(W_qkv, np.float64)
    bqkv64 = np.asarray(b_qkv, np.float64)
    Wag64 = np.asarray(W_agent, np.float64)
    bag64 = np.asarray(b_agent, np.float64)
    Wf1 = np.asarray(W_fc1, np.float64)
    bf1 = np.asarray(b_fc1, np.float64)
    Wf2 = np.asarray(W_fc2, np.float64)
    bf2 = np.asarray(b_fc2, np.float64)

    ag = agent64 @ Wag64 + bag64
    q_agent, k_agent = ag[:, :D], ag[:, D:]
    W_q, W_k, W_v = Wqkv64[:, :D], Wqkv64[:, D : 2 * D], Wqkv64[:, 2 * D :]
    b_q, b_v = bqkv64[:D], bqkv64[2 * D :]

    wk_f = W_k @ q_agent.T                      # [D, K]
    wq_f = W_q @ k_agent.T                      # [D, K]
    c2_f = (D ** -0.5) * (k_agent @ b_q)        # [K]
    ec2_f = np.exp(c2_f)                        # [K]
    Wbig = W_v @ Wf1 @ Wf2                      # [D, D]
    bbig = (b_v @ Wf1 + bf1) @ Wf2 + bf2        # [D], added on host

    # [D, D] -> [P, DS, D] with d = s*128 + p
    wcombo_b = np.ascontiguousarray(
        Wbig.reshape(DS, P, D).transpose(1, 0, 2)
    ).astype(_BF16)
    wkq8 = np.concatenate([wk_f, wq_f], axis=1).reshape(DS, P, 2 * K)
    wkq8 = np.ascontiguousarray(wkq8.transpose(1, 0, 2)).astype(_FP8)
    fcombo = np.zeros((K, 1 + K), np.float32)
    fcombo[:, 0] = ec2_f
    fcombo[:, 1 :] = np.eye(K)
    fcombo = np.ascontiguousarray(fcombo).astype(_BF16)

    x32 = np.asarray(x, np.float32)
    # xe pack: [B, N, E] -> [B, P, NT, E], token = t*128 + p
    xb = np.ones((B, N, E), _FP8)
    xb[:, :, :D] = x32.astype(_FP8)
    xeb = np.ascontiguousarray(xb.reshape(B, NT, P, E).transpose(0, 2, 1, 3))
    # xT pack: [B, D, N] -> [B, P, DS, N], d = s*128 + p
    xTb = x32.transpose(0, 2, 1).reshape(B, DS, P, N)
    xTb = np.ascontiguousarray(xTb.transpose(0, 2, 1, 3)).astype(_FP8)

    in_maps = [
        {
            "xT": xTb[i],
            "xe": xeb[i],
            "wkq8": wkq8,
            "wcombo": wcombo_b,
            "fcombo": fcombo,
        }
        for i in range(B)
    ]

    return in_maps, x32, bbig.astype(np.float32)


def kernel(**inputs):
    from concourse.bass_utils import run_bass_kernel_spmd

    in_maps, x32, bbig = _prepare_in_maps(**inputs)
    nc = _get_nc()
    res_obj = run_bass_kernel_spmd(nc, in_maps, core_ids=list(range(B)))
    _CACHE["last_results"] = res_obj
    res = res_obj.results

    # ye [P, NT, E] -> [N, E] with token = t*128 + p
    ye = np.stack(
        [np.asarray(res[i]["ye"]).transpose(1, 0, 2).reshape(N, E) for i in range(B)]
    ).astype(np.float32)
    out = ye[:, :, :D] / ye[:, :, D:E] + bbig[None, None, :] + x32
    return out.astype(np.float32)


# revision 9
# speedup vs baseline: 1.1786x; 1.0051x over previous
"""Trainium2 Bass kernel for agent-attention (AAGA): 8-core data-parallel over batch.

Math (per batch b):
  qkv = x @ W_qkv + b_qkv ; q,k,v = split(qkv)
  ag  = agent @ W_agent + b_agent ; q_agent,k_agent = split(ag)
  attn1 = softmax(q_agent @ k^T * s)        # [K, N]
  va    = (attn1 @ v) @ W_fc1 + b_fc1       # [K, d]
  attn2 = softmax(q @ k_agent^T * s)        # [N, K]
  out   = (attn2 @ va) @ W_fc2 + b_fc2 + x  # [N, d]

Host-side algebraic folds (everything not involving x is an input):
  q_agent/k_agent computed on host; q,k,v never materialized on device.
  S1^T = x @ (W_k@q_agent^T): b_k drops out of the softmax (shift invariance).
  va-chain: attn1 rows sum to 1, so all later biases fold into a single
       constant row bbig = (b_v@W_fc1+b_fc1)@W_fc2 + b_fc2 ADDED ON HOST.
  Device vaF[k,:] = [ (ec2/s1)[k] * (expS1^T x)[k,:] @ Wbig | ec2[k] ],
  with Wbig = W_v@W_fc1@W_fc2. Then y = expS2^T @ vaF gives
  y[:, :D] = attn2-numerator combination and y[:, D] = s2 (denominator).
  Host epilogue: out = y[:, :D]/y[:, D] + bbig + x  (exact fp32).

DMA/engine regime (cost model): each DMACopy costs ~625ns on a single
serialized HWDGE queue (SP/Act) or ~1us on the Pool engine via SWDGE;
transfers serialize on DMA_ENGINES at 360GB/s with a 2x penalty for
runs <512B. GPSIMD cannot touch PSUM, so all PSUM->SBUF casts go on
DVE+Act, batched 3-4 token-tiles per instruction via multi-bank PSUM
tiles, alternating two PSUM pools so matmul and copy pipeline.
"""

import numpy as np
import ml_dtypes

B, N, D, K = 8, 4096, 256, 64
E = D + 1          # ones-column appended
P = 128
NT = N // P        # 32 token tiles
DS = D // P        # 2 contraction subtiles
W = 512            # free-dim chunk for S2^T
NC2 = N // W       # 8 chunks

# input streaming chunks (in token tiles); last chunks small to cut tail lag
XCHUNKS = [8, 8, 8, 6, 2]
# S1 slabs (in token tiles); small last slabs shorten the avx tail
SLABS = [8, 8, 8, 4, 2, 2]
# y-phase groups (in token tiles); small last groups shorten the DMA tail
YGROUPS = [4, 4, 4, 4, 4, 4, 4, 2, 2]

_BF16 = ml_dtypes.bfloat16
_FP8 = ml_dtypes.float8_e4m3

_CACHE = {}


def _build_nc():
    import concourse.bass as bass
    import concourse.tile as tile
    from concourse import bacc, mybir

    f32 = mybir.dt.float32
    bf16 = mybir.dt.bfloat16
    fp8 = mybir.dt.float8e4
    Exp = mybir.ActivationFunctionType.Exp
    DR = mybir.MatmulPerfMode.DoubleRow
    Copy = mybir.ActivationFunctionType.Copy
    ts = bass.ts

    nc = bacc.Bacc("TRN2", target_bir_lowering=False, debug=False)

    xT_d = nc.declare_dram_parameter("xT", [P, DS, N], fp8, isOutput=False)
    xe_d = nc.declare_dram_parameter("xe", [P, NT, E], fp8, isOutput=False)
    wkq8_d = nc.declare_dram_parameter("wkq8", [P, DS, 2 * K], fp8, isOutput=False)
    wcombo_d = nc.declare_dram_parameter("wcombo", [P, DS, D], bf16, isOutput=False)
    FC = 1 + K         # [ec2 | I64]
    fcombo_d = nc.declare_dram_parameter("fcombo", [K, FC], bf16, isOutput=False)
    ye_d = nc.declare_dram_parameter("ye", [P, NT, E], fp8, isOutput=True)

    with tile.TileContext(nc) as tc:
        with (
            tc.tile_pool(name="sb", bufs=1) as sb,
            tc.tile_pool(name="yout", bufs=4) as yout,
        ):
            # ---------------- input DMAs ----------------
            wkq8 = sb.tile([P, DS, 2 * K], fp8)
            nc.sync.dma_start(out=wkq8, in_=wkq8_d[:, :, :])
            wk = wkq8[:, :, 0:K]
            wq = wkq8[:, :, K : 2 * K]

            xT = sb.tile([P, DS, N], fp8)
            xe = sb.tile([P, NT, E], fp8)
            t0 = 0
            for ci, ct in enumerate(XCHUNKS):
                nc.sync.dma_start(
                    out=xT[:, :, P * t0 : P * (t0 + ct)],
                    in_=xT_d[:, :, P * t0 : P * (t0 + ct)],
                )
                nc.gpsimd.dma_start(
                    out=xe[:, t0 : t0 + ct, :], in_=xe_d[:, t0 : t0 + ct, :]
                )
                if ci == 2:
                    wcombo = sb.tile([P, DS, D], bf16)
                    nc.sync.dma_start(out=wcombo, in_=wcombo_d[:, :, :])
                    fcombo = sb.tile([K, FC], bf16)
                    nc.sync.dma_start(out=fcombo, in_=fcombo_d[:, :])
                t0 += ct
            ec2 = fcombo[:, 0:1]
            ident = fcombo[:, 1 : 1 + K]            # I64 bf16

            expS1 = sb.tile([P, NT, K], fp8)    # token-major exp(S1), /16-shifted
            sh1 = sb.tile([P, 1], f32)
            nc.vector.memset(sh1, -2.772588722239781)   # -ln(16): keeps exp < 240 (fp8 max)
            # dummy exp: pulls the 1.3us LoadActFuncSet into the DMA head
            warm = sb.tile([P, 1], f32)
            nc.scalar.activation(warm, sh1, Exp)
            expS2 = sb.tile([K, NC2, W], bf16)          # agent-major exp(S2)

            # vaF holder; constant ec2 column written early
            vaF = sb.tile([K, E], bf16)
            nc.vector.tensor_copy(vaF[:, D:E], ec2)

            # ---- phase 1: S1 slabs + free-axis-paired S2 chunks ----
            # deep PSUM rings so matmul(n+1) never waits exp(n); scoped so the
            # y phase can reuse all 8 banks afterwards
            with (
                tc.tile_pool(name="s1p", bufs=3, space="PSUM") as s1p,  # 3 banks
                tc.tile_pool(name="s2p", bufs=2, space="PSUM") as s2p,  # 4 banks
                tc.tile_pool(name="pX", bufs=1, space="PSUM") as pX,    # 1 bank
            ):
                avx_ps = pX.tile([K, E], f32, tag="pX")
                nslab = len(SLABS)
                sstart = [sum(SLABS[:i]) for i in range(nslab)]

                def s1_slab(b):
                    t0, sl = sstart[b], SLABS[b]
                    ps = s1p.tile([P, sl, K], f32, tag="s1p")
                    for j in range(sl):
                        t = t0 + j
                        # DoubleRow: 2 fp8 weights/cell -> 256-contraction in one mm
                        nc.tensor.matmul(
                            ps[:, j, :], xT[:, :, ts(t, P)], wk,
                            start=True, stop=True, perf_mode=DR,
                        )
                    nc.scalar.activation(
                        expS1[:, t0 : t0 + sl, :], ps, Exp,
                        scale=float(D ** -0.5), bias=sh1,
                    )
                    for j in range(sl // 2):
                        u = t0 // 2 + j
                        nc.tensor.matmul(
                            avx_ps, expS1[:, 2 * u : 2 * u + 2, :],
                            xe[:, 2 * u : 2 * u + 2, :],
                            start=(u == 0), stop=(u == NT // 2 - 1), perf_mode=DR,
                        )

                def s2_pair(h):
                    # two 512-token chunks in two PSUM banks -> one exp, free 1024
                    p2 = s2p.tile([K, 2, W], f32, tag="s2p")
                    for g in range(2):
                        c = 2 * h + g
                        nc.tensor.matmul(
                            p2[:, g, :], wq, xT[:, :, ts(c, W)],
                            start=True, stop=True, perf_mode=DR,
                        )
                    nc.scalar.activation(
                        expS2[:, 2 * h : 2 * h + 2, :], p2, Exp,
                        scale=float(D ** -0.5), bias=sh1[0:K, :],
                    )

                for b in range(nslab):
                    s1_slab(b)
                # S2 logits only feed the y phase; running them after the S1
                # stream keeps Act free so expS1 (which gates avx/vaF) never
                # queues behind a 1us S2 exp. They overlap the vaF chain.
                for h in range(NC2 // 2):
                    s2_pair(h)

                # ---- vaF[:, :D] = (avx*ec2/s1 @ Wbig) ----
                rec1 = sb.tile([K, 1], f32)
                nc.vector.reciprocal(rec1, avx_ps[:, D:E])
                nc.vector.tensor_tensor(rec1, rec1, ec2, mybir.AluOpType.mult)
                avx_s = sb.tile([K, D], bf16)
                nc.vector.tensor_scalar_mul(avx_s, avx_ps[:, :D], rec1)
                avxT = sb.tile([P, DS, K], bf16)
                tp = s1p.tile([P, DS, K], bf16, tag="s1p")
                for s in range(DS):
                    nc.tensor.transpose(tp[:, s, :], avx_s[:, ts(s, P)], ident)
                nc.vector.tensor_copy(avxT[:, 0, :], tp[:, 0, :])
                nc.scalar.activation(avxT[:, 1, :], tp[:, 1, :], Copy)
                vf_ps = s1p.tile([K, D], f32, tag="s1p")
                for s in range(DS):
                    nc.tensor.matmul(
                        vf_ps, avxT[:, s, :], wcombo[:, s, :],
                        start=(s == 0), stop=(s == DS - 1),
                    )
                nc.vector.tensor_copy(vaF[:, 0 : D // 2], vf_ps[:, 0 : D // 2])
                nc.scalar.activation(vaF[:, D // 2 : D], vf_ps[:, D // 2 : D], Copy)

            # ---- y_ext[n, :] = sum_k expS2[k,n] * vaF_ext[k, :] ----
            # col D of vaF_ext is ec2, so col D of y_ext = s2. 4-tile groups in
            # a bufs=2 pool (4 banks each) so matmul and cast pipeline; casts
            # alternate DVE/Act; one DMA out per group.
            with tc.tile_pool(name="ypool", bufs=2, space="PSUM") as ypool:
                g0 = 0
                for gi, gsz in enumerate(YGROUPS):
                    yp = ypool.tile([P, 4, W], f32, tag="ypool")
                    y_sb = yout.tile([P, gsz, E], fp8, tag="ysb")
                    for j in range(gsz):
                        t = g0 + j
                        nc.tensor.matmul(
                            yp[:, j, 0:E],
                            expS2[:, t // 4, ts(t % 4, P)],
                            vaF, start=True, stop=True,
                        )
                    if gi < 2 or gi % 2 == 0:
                        nc.vector.tensor_copy(y_sb, yp[:, 0:gsz, 0:E])
                    else:
                        nc.scalar.activation(y_sb, yp[:, 0:gsz, 0:E], Copy)
                    nc.sync.dma_start(
                        out=ye_d[:, g0 : g0 + gsz, :], in_=y_sb
                    )
                    g0 += gsz

    nc.compile()
    return nc


def _get_nc():
    if "nc" not in _CACHE:
        _CACHE["nc"] = _build_nc()
    return _CACHE["nc"]


def _prepare_in_maps(agent, x, W_qkv, b_qkv, W_agent, b_agent, W_fc1, b_fc1, W_fc2, b_fc2):
    # ---- host folds (float64 for stability, cast down at the end) ----
    agent64 = np.asarray(agent, np.float64)
    Wqkv64 = np.asarray(W_qkv, np.float64)
    bqkv64 = np.asarray(b_qkv, np.float64)
    Wag64 = np.asarray(W_agent, np.float64)
    bag64 = np.asarray(b_agent, np.float64)
    Wf1 = np.asarray(W_fc1, np.float64)
    bf1 = np.asarray(b_fc1, np.float64)
    Wf2 = np.asarray(W_fc2, np.float64)
    bf2 = np.asarray(b_fc2, np.float64)

    ag = agent64 @ Wag64 + bag64
    q_agent, k_agent = ag[:, :D], ag[:, D:]
    W_q, W_k, W_v = Wqkv64[:, :D], Wqkv64[:, D : 2 * D], Wqkv64[:, 2 * D :]
    b_q, b_v = bqkv64[:D], bqkv64[2 * D :]

    wk_f = W_k @ q_agent.T                      # [D, K]
    wq_f = W_q @ k_agent.T                      # [D, K]
    c2_f = (D ** -0.5) * (k_agent @ b_q)        # [K]
    ec2_f = np.exp(c2_f)                        # [K]
    Wbig = W_v @ Wf1 @ Wf2                      # [D, D]
    bbig = (b_v @ Wf1 + bf1) @ Wf2 + bf2        # [D], added on host

    # [D, D] -> [P, DS, D] with d = s*128 + p
    wcombo_b = np.ascontiguousarray(
        Wbig.reshape(DS, P, D).transpose(1, 0, 2)
    ).astype(_BF16)
    wkq8 = np.concatenate([wk_f, wq_f], axis=1).reshape(DS, P, 2 * K)
    wkq8 = np.ascontiguousarray(wkq8.transpose(1, 0, 2)).astype(_FP8)
    fcombo = np.zeros((K, 1 + K), np.float32)
    fcombo[:, 0] = ec2_f
    fcombo[:, 1 :] = np.eye(K)
    fcombo = np.ascontiguousarray(fcombo).astype(_BF16)

    x32 = np.asarray(x, np.float32)
    # xe pack: [B, N, E] -> [B, P, NT, E], token = t*128 + p
    xb = np.ones((B, N, E), _FP8)
    xb[:, :, :D] = x32.astype(_FP8)
    xeb = np.ascontiguousarray(xb.reshape(B, NT, P, E).transpose(0, 2, 1, 3))
    # xT pack: [B, D, N] -> [B, P, DS, N], d = s*128 + p
    xTb = x32.transpose(0, 2, 1).reshape(B, DS, P, N)
    xTb = np.ascontiguousarray(xTb.transpose(0, 2, 1, 3)).astype(_FP8)

    in_maps = [
        {
            "xT": xTb[i],
            "xe": xeb[i],
            "wkq8": wkq8,
            "wcombo": wcombo_b,
            "fcombo": fcombo,
        }
        for i in range(B)
    ]

    return in_maps, x32, bbig.astype(np.float32)


def kernel(**inputs):
    from concourse.bass_utils import run_bass_kernel_spmd

    in_maps, x32, bbig = _prepare_in_maps(**inputs)
    nc = _get_nc()
    res_obj = run_bass_kernel_spmd(nc, in_maps, core_ids=list(range(B)))
    _CACHE["last_results"] = res_obj
    res = res_obj.results

    # ye [P, NT, E] -> [N, E] with token = t*128 + p
    ye = np.stack(
        [np.asarray(res[i]["ye"]).transpose(1, 0, 2).reshape(N, E) for i in range(B)]
    ).astype(np.float32)
    out = ye[:, :, :D] / ye[:, :, D:E] + bbig[None, None, :] + x32
    return out.astype(np.float32)


# revision 11
# speedup vs baseline: 1.2553x; 1.0651x over previous
"""Trainium2 Bass kernel for agent-attention (AAGA): 8-core data-parallel over batch.

Math (per batch b):
  qkv = x @ W_qkv + b_qkv ; q,k,v = split(qkv)
  ag  = agent @ W_agent + b_agent ; q_agent,k_agent = split(ag)
  attn1 = softmax(q_agent @ k^T * s)        # [K, N]
  va    = (attn1 @ v) @ W_fc1 + b_fc1       # [K, d]
  attn2 = softmax(q @ k_agent^T * s)        # [N, K]
  out   = (attn2 @ va) @ W_fc2 + b_fc2 + x  # [N, d]

Host-side algebraic folds (everything not involving x is an input):
  q_agent/k_agent computed on host; q,k,v never materialized on device.
  S1^T = x @ (W_k@q_agent^T): b_k drops out of the softmax (shift invariance).
  va-chain: attn1 rows sum to 1, so all later biases fold into a single
       constant row bbig = (b_v@W_fc1+b_fc1)@W_fc2 + b_fc2 ADDED ON HOST.
  Device vaF[k,:] = [ (ec2/s1)[k] * (expS1^T x)[k,:] @ Wbig | ec2[k] ],
  with Wbig = W_v@W_fc1@W_fc2. Then y = expS2^T @ vaF gives
  y[:, :D] = attn2-numerator combination and y[:, D] = s2 (denominator).
  Host epilogue: out = y[:, :D]/y[:, D] + bbig + x  (exact fp32).

DMA/engine regime (cost model): each DMACopy costs ~625ns on a single
serialized HWDGE queue (SP/Act) or ~1us on the Pool engine via SWDGE;
transfers serialize on DMA_ENGINES at 360GB/s with a 2x penalty for
runs <512B. GPSIMD cannot touch PSUM, so all PSUM->SBUF casts go on
DVE+Act, batched 3-4 token-tiles per instruction via multi-bank PSUM
tiles, alternating two PSUM pools so matmul and copy pipeline.
"""

import numpy as np
import ml_dtypes

B, N, D, K = 8, 4096, 256, 64
E = D + 1          # ones-column appended
P = 128
NT = N // P        # 32 token tiles
DS = D // P        # 2 contraction subtiles
W = 512            # free-dim chunk for S2^T
NC2 = N // W       # 8 chunks

# input streaming chunks (in token tiles); small first chunk starts compute
# early, small last chunk cuts the avx tail
XCHUNKS = [4, 8, 8, 8, 4]
# S1 slabs (in token tiles); small last slabs shorten the avx tail
SLABS = [4, 8, 8, 8, 2, 2]
# y-phase groups (in token tiles); small last groups shorten the DMA tail
YGROUPS = [4, 4, 4, 4, 4, 4, 4, 2, 2]

_BF16 = ml_dtypes.bfloat16
_FP8 = ml_dtypes.float8_e4m3

_CACHE = {}


def _build_nc():
    import concourse.bass as bass
    import concourse.tile as tile
    from concourse import bacc, mybir

    f32 = mybir.dt.float32
    bf16 = mybir.dt.bfloat16
    fp8 = mybir.dt.float8e4
    Exp = mybir.ActivationFunctionType.Exp
    DR = mybir.MatmulPerfMode.DoubleRow
    Copy = mybir.ActivationFunctionType.Copy
    ts = bass.ts

    nc = bacc.Bacc("TRN2", target_bir_lowering=False, debug=False)

    xT_d = nc.declare_dram_parameter("xT", [P, DS, N], fp8, isOutput=False)
    xe_d = nc.declare_dram_parameter("xe", [P, NT, E], fp8, isOutput=False)
    wkq8_d = nc.declare_dram_parameter("wkq8", [P, DS, 2 * K], fp8, isOutput=False)
    wcombo_d = nc.declare_dram_parameter("wcombo", [P, DS, D], bf16, isOutput=False)
    FC = 1 + K         # [ec2 | I64]
    fcombo_d = nc.declare_dram_parameter("fcombo", [K, FC], bf16, isOutput=False)
    ye_d = nc.declare_dram_parameter("ye", [P, NT, E], fp8, isOutput=True)

    with tile.TileContext(nc) as tc:
        with (
            tc.tile_pool(name="sb", bufs=1) as sb,
            tc.tile_pool(name="yout", bufs=4) as yout,
        ):
            # ---------------- input DMAs ----------------
            wkq8 = sb.tile([P, DS, 2 * K], fp8)
            nc.sync.dma_start(out=wkq8, in_=wkq8_d[:, :, :])
            wk = wkq8[:, :, 0:K]
            wq = wkq8[:, :, K : 2 * K]

            xT = sb.tile([P, DS, N], fp8)
            xe = sb.tile([P, NT, E], fp8)
            t0 = 0
            for ci, ct in enumerate(XCHUNKS):
                nc.sync.dma_start(
                    out=xT[:, :, P * t0 : P * (t0 + ct)],
                    in_=xT_d[:, :, P * t0 : P * (t0 + ct)],
                )
                nc.gpsimd.dma_start(
                    out=xe[:, t0 : t0 + ct, :], in_=xe_d[:, t0 : t0 + ct, :]
                )
                if ci == 1:
                    wcombo = sb.tile([P, DS, D], bf16)
                    nc.sync.dma_start(out=wcombo, in_=wcombo_d[:, :, :])
                    fcombo = sb.tile([K, FC], bf16)
                    nc.sync.dma_start(out=fcombo, in_=fcombo_d[:, :])
                t0 += ct
            ec2 = fcombo[:, 0:1]
            ident = fcombo[:, 1 : 1 + K]            # I64 bf16

            expS1 = sb.tile([P, NT, K], fp8)    # token-major exp(S1), /16-shifted
            sh1 = sb.tile([P, 1], f32)
            nc.vector.memset(sh1, -2.772588722239781)   # -ln(16): keeps exp < 240 (fp8 max)
            # dummy exp: pulls the 1.3us LoadActFuncSet into the DMA head
            warm = sb.tile([P, 1], f32)
            nc.scalar.activation(warm, sh1, Exp)
            expS2 = sb.tile([K, NC2, W], bf16)          # agent-major exp(S2)

            # vaF holder; constant ec2 column written early
            vaF = sb.tile([K, E], bf16)
            nc.vector.tensor_copy(vaF[:, D:E], ec2)

            # ---- phase 1: S1 slabs + free-axis-paired S2 chunks ----
            # deep PSUM rings so matmul(n+1) never waits exp(n); scoped so the
            # y phase can reuse all 8 banks afterwards
            with (
                tc.tile_pool(name="s1p", bufs=3, space="PSUM") as s1p,  # 3 banks
                tc.tile_pool(name="s2p", bufs=2, space="PSUM") as s2p,  # 4 banks
                tc.tile_pool(name="pX", bufs=1, space="PSUM") as pX,    # 1 bank
            ):
                avx_ps = pX.tile([K, E], f32, tag="pX")
                nslab = len(SLABS)
                sstart = [sum(SLABS[:i]) for i in range(nslab)]

                def s1_slab(b):
                    t0, sl = sstart[b], SLABS[b]
                    ps = s1p.tile([P, sl, K], f32, tag="s1p")
                    for j in range(sl):
                        t = t0 + j
                        # DoubleRow: 2 fp8 weights/cell -> 256-contraction in one mm
                        nc.tensor.matmul(
                            ps[:, j, :], xT[:, :, ts(t, P)], wk,
                            start=True, stop=True, perf_mode=DR,
                        )
                    nc.scalar.activation(
                        expS1[:, t0 : t0 + sl, :], ps, Exp,
                        scale=float(D ** -0.5), bias=sh1,
                    )
                    for j in range(sl // 2):
                        u = t0 // 2 + j
                        nc.tensor.matmul(
                            avx_ps, expS1[:, 2 * u : 2 * u + 2, :],
                            xe[:, 2 * u : 2 * u + 2, :],
                            start=(u == 0), stop=(u == NT // 2 - 1), perf_mode=DR,
                        )

                def s2_pair(h):
                    # two 512-token chunks in two PSUM banks -> one exp, free 1024
                    p2 = s2p.tile([K, 2, W], f32, tag="s2p")
                    for g in range(2):
                        c = 2 * h + g
                        nc.tensor.matmul(
                            p2[:, g, :], wq, xT[:, :, ts(c, W)],
                            start=True, stop=True, perf_mode=DR,
                        )
                    nc.scalar.activation(
                        expS2[:, 2 * h : 2 * h + 2, :], p2, Exp,
                        scale=float(D ** -0.5), bias=sh1[0:K, :],
                    )

                for b in range(nslab):
                    s1_slab(b)
                # S2 logits only feed the y phase; running them after the S1
                # stream keeps Act free so expS1 (which gates avx/vaF) never
                # queues behind a 1us S2 exp. They overlap the vaF chain.
                for h in range(NC2 // 2):
                    s2_pair(h)

                # ---- vaF[:, :D] = (avx*ec2/s1 @ Wbig) ----
                rec1 = sb.tile([K, 1], f32)
                nc.vector.reciprocal(rec1, avx_ps[:, D:E])
                nc.vector.tensor_tensor(rec1, rec1, ec2, mybir.AluOpType.mult)
                avx_s = sb.tile([K, D], bf16)
                nc.vector.tensor_scalar_mul(avx_s, avx_ps[:, :D], rec1)
                avxT = sb.tile([P, DS, K], bf16)
                tp = s1p.tile([P, DS, K], bf16, tag="s1p")
                for s in range(DS):
                    nc.tensor.transpose(tp[:, s, :], avx_s[:, ts(s, P)], ident)
                nc.vector.tensor_copy(avxT, tp)    # bf16: DVE 2x mode
                vf_ps = s1p.tile([K, D], f32, tag="s1p")
                for s in range(DS):
                    nc.tensor.matmul(
                        vf_ps, avxT[:, s, :], wcombo[:, s, :],
                        start=(s == 0), stop=(s == DS - 1),
                    )
                nc.vector.tensor_copy(vaF[:, 0:D], vf_ps)

            # ---- y_ext[n, :] = sum_k expS2[k,n] * vaF_ext[k, :] ----
            # col D of vaF_ext is ec2, so col D of y_ext = s2. 4-tile groups in
            # a bufs=2 pool pipeline matmul vs cast; casts rotate 2:1 DVE:Act
            # (Act is still finishing S2 exps when the early groups land).
            with tc.tile_pool(name="ypool", bufs=2, space="PSUM") as ypool:
                g0 = 0
                y_sb = None
                for gi, gsz in enumerate(YGROUPS):
                    yp = ypool.tile([P, 4, W], f32, tag="ypool")
                    if gi % 2 == 0:
                        ysz = gsz if gi == len(YGROUPS) - 1 else gsz + YGROUPS[gi + 1]
                        y_sb = yout.tile([P, ysz, E], fp8, tag="ysb")
                        ysb0 = g0
                    for j in range(gsz):
                        t = g0 + j
                        nc.tensor.matmul(
                            yp[:, j, 0:E],
                            expS2[:, t // 4, ts(t % 4, P)],
                            vaF, start=True, stop=True,
                        )
                    dst = y_sb[:, g0 - ysb0 : g0 - ysb0 + gsz, :]
                    if gi % 3 == 2:
                        nc.scalar.activation(dst, yp[:, 0:gsz, 0:E], Copy)
                    else:
                        nc.vector.tensor_copy(dst, yp[:, 0:gsz, 0:E])
                    if gi % 2 == 1 or gi == len(YGROUPS) - 1:
                        qn = g0 + gsz - ysb0
                        nc.sync.dma_start(
                            out=ye_d[:, ysb0 : ysb0 + qn, :], in_=y_sb[:, 0:qn, :]
                        )
                    g0 += gsz

    nc.compile()
    return nc


def _get_nc():
    if "nc" not in _CACHE:
        _CACHE["nc"] = _build_nc()
    return _CACHE["nc"]


def _prepare_in_maps(agent, x, W_qkv, b_qkv, W_agent, b_agent, W_fc1, b_fc1, W_fc2, b_fc2):
    # ---- host folds (float64 for stability, cast down at the end) ----
    agent64 = np.asarray(agent, np.float64)
    Wqkv64 = np.asarray(W_qkv, np.float64)
    bqkv64 = np.asarray(b_qkv, np.float64)
    Wag64 = np.asarray(W_agent, np.float64)
    bag64 = np.asarray(b_agent, np.float64)
    Wf1 = np.asarray(W_fc1, np.float64)
    bf1 = np.asarray(b_fc1, np.float64)
    Wf2 = np.asarray(W_fc2, np.float64)
    bf2 = np.asarray(b_fc2, np.float64)

    ag = agent64 @ Wag64 + bag64
    q_agent, k_agent = ag[:, :D], ag[:, D:]
    W_q, W_k, W_v = Wqkv64[:, :D], Wqkv64[:, D : 2 * D], Wqkv64[:, 2 * D :]
    b_q, b_v = bqkv64[:D], bqkv64[2 * D :]

    wk_f = W_k @ q_agent.T                      # [D, K]
    wq_f = W_q @ k_agent.T                      # [D, K]
    c2_f = (D ** -0.5) * (k_agent @ b_q)        # [K]
    ec2_f = np.exp(c2_f)                        # [K]
    Wbig = W_v @ Wf1 @ Wf2                      # [D, D]
    bbig = (b_v @ Wf1 + bf1) @ Wf2 + bf2        # [D], added on host

    # [D, D] -> [P, DS, D] with d = s*128 + p
    wcombo_b = np.ascontiguousarray(
        Wbig.reshape(DS, P, D).transpose(1, 0, 2)
    ).astype(_BF16)
    wkq8 = np.concatenate([wk_f, wq_f], axis=1).reshape(DS, P, 2 * K)
    wkq8 = np.ascontiguousarray(wkq8.transpose(1, 0, 2)).astype(_FP8)
    fcombo = np.zeros((K, 1 + K), np.float32)
    fcombo[:, 0] = ec2_f
    fcombo[:, 1 :] = np.eye(K)
    fcombo = np.ascontiguousarray(fcombo).astype(_BF16)

    x32 = np.asarray(x, np.float32)
    # xe pack: [B, N, E] -> [B, P, NT, E], token = t*128 + p
    xb = np.ones((B, N, E), _FP8)
    xb[:, :, :D] = x32.astype(_FP8)
    xeb = np.ascontiguousarray(xb.reshape(B, NT, P, E).transpose(0, 2, 1, 3))
    # xT pack: [B, D, N] -> [B, P, DS, N], d = s*128 + p
    xTb = x32.transpose(0, 2, 1).reshape(B, DS, P, N)
    xTb = np.ascontiguousarray(xTb.transpose(0, 2, 1, 3)).astype(_FP8)

    in_maps = [
        {
            "xT": xTb[i],
            "xe": xeb[i],
            "wkq8": wkq8,
            "wcombo": wcombo_b,
            "fcombo": fcombo,
        }
        for i in range(B)
    ]

    return in_maps, x32, bbig.astype(np.float32)


def kernel(**inputs):
    from concourse.bass_utils import run_bass_kernel_spmd

    in_maps, x32, bbig = _prepare_in_maps(**inputs)
    nc = _get_nc()
    res_obj = run_bass_kernel_spmd(nc, in_maps, core_ids=list(range(B)))
    _CACHE["last_results"] = res_obj
    res = res_obj.results

    # ye [P, NT, E] -> [N, E] with token = t*128 + p
    ye = np.stack(
        [np.asarray(res[i]["ye"]).transpose(1, 0, 2).reshape(N, E) for i in range(B)]
    ).astype(np.float32)
    out = ye[:, :, :D] / ye[:, :, D:E] + bbig[None, None, :] + x32
    return out.astype(np.float32)


# revision 12
# speedup vs baseline: 1.2694x; 1.0113x over previous
"""Trainium2 Bass kernel for agent-attention (AAGA): 8-core data-parallel over batch.

Math (per batch b):
  qkv = x @ W_qkv + b_qkv ; q,k,v = split(qkv)
  ag  = agent @ W_agent + b_agent ; q_agent,k_agent = split(ag)
  attn1 = softmax(q_agent @ k^T * s)        # [K, N]
  va    = (attn1 @ v) @ W_fc1 + b_fc1       # [K, d]
  attn2 = softmax(q @ k_agent^T * s)        # [N, K]
  out   = (attn2 @ va) @ W_fc2 + b_fc2 + x  # [N, d]

Host-side algebraic folds (everything not involving x is an input):
  q_agent/k_agent computed on host; q,k,v never materialized on device.
  S1^T = x @ (W_k@q_agent^T): b_k drops out of the softmax (shift invariance).
  va-chain: attn1 rows sum to 1, so all later biases fold into a single
       constant row bbig = (b_v@W_fc1+b_fc1)@W_fc2 + b_fc2 ADDED ON HOST.
  Device vaF[k,:] = [ (ec2/s1)[k] * (expS1^T x)[k,:] @ Wbig | ec2[k] ],
  with Wbig = W_v@W_fc1@W_fc2. Then y = expS2^T @ vaF gives
  y[:, :D] = attn2-numerator combination and y[:, D] = s2 (denominator).
  Host epilogue: out = y[:, :D]/y[:, D] + bbig + x  (exact fp32).

DMA/engine regime (cost model): each DMACopy costs ~625ns on a single
serialized HWDGE queue (SP/Act) or ~1us on the Pool engine via SWDGE;
transfers serialize on DMA_ENGINES at 360GB/s with a 2x penalty for
runs <512B. GPSIMD cannot touch PSUM, so all PSUM->SBUF casts go on
DVE+Act, batched 3-4 token-tiles per instruction via multi-bank PSUM
tiles, alternating two PSUM pools so matmul and copy pipeline.
"""

import numpy as np
import ml_dtypes

B, N, D, K = 8, 4096, 256, 64
E = D + 1          # ones-column appended
P = 128
NT = N // P        # 32 token tiles
DS = D // P        # 2 contraction subtiles
W = 512            # free-dim chunk for S2^T
NC2 = N // W       # 8 chunks

# input streaming chunks (in token tiles); small first chunk starts compute
# early, small last chunk cuts the avx tail
XCHUNKS = [4, 8, 8, 8, 4]
# S1 slabs (in token tiles); small last slab shortens the avx tail
SLABS = [4, 8, 8, 8, 4]
# y-phase groups (in token tiles); small last groups shorten the DMA tail
YGROUPS = [4, 4, 4, 4, 4, 4, 4, 2, 2]

_BF16 = ml_dtypes.bfloat16
_FP8 = ml_dtypes.float8_e4m3

_CACHE = {}


def _build_nc():
    import concourse.bass as bass
    import concourse.tile as tile
    from concourse import bacc, mybir

    f32 = mybir.dt.float32
    bf16 = mybir.dt.bfloat16
    fp8 = mybir.dt.float8e4
    Exp = mybir.ActivationFunctionType.Exp
    DR = mybir.MatmulPerfMode.DoubleRow
    Copy = mybir.ActivationFunctionType.Copy
    ts = bass.ts

    nc = bacc.Bacc("TRN2", target_bir_lowering=False, debug=False)

    xT_d = nc.declare_dram_parameter("xT", [P, DS, N], fp8, isOutput=False)
    xe_d = nc.declare_dram_parameter("xe", [P, NT, E], fp8, isOutput=False)
    wkq8_d = nc.declare_dram_parameter("wkq8", [P, DS, 2 * K], fp8, isOutput=False)
    wcombo_d = nc.declare_dram_parameter("wcombo", [P, DS, D], bf16, isOutput=False)
    FC = 1 + K         # [ec2 | I64]
    fcombo_d = nc.declare_dram_parameter("fcombo", [K, FC], bf16, isOutput=False)
    ye_d = nc.declare_dram_parameter("ye", [P, NT, E], fp8, isOutput=True)

    with tile.TileContext(nc) as tc:
        with (
            tc.tile_pool(name="sb", bufs=1) as sb,
            tc.tile_pool(name="yout", bufs=4) as yout,
        ):
            # ---------------- input DMAs ----------------
            wkq8 = sb.tile([P, DS, 2 * K], fp8)
            nc.sync.dma_start(out=wkq8, in_=wkq8_d[:, :, :])
            wk = wkq8[:, :, 0:K]
            wq = wkq8[:, :, K : 2 * K]

            xT = sb.tile([P, DS, N], fp8)
            xe = sb.tile([P, NT, E], fp8)
            t0 = 0
            for ci, ct in enumerate(XCHUNKS):
                nc.sync.dma_start(
                    out=xT[:, :, P * t0 : P * (t0 + ct)],
                    in_=xT_d[:, :, P * t0 : P * (t0 + ct)],
                )
                nc.gpsimd.dma_start(
                    out=xe[:, t0 : t0 + ct, :], in_=xe_d[:, t0 : t0 + ct, :]
                )
                if ci == 1:
                    wcombo = sb.tile([P, DS, D], bf16)
                    nc.sync.dma_start(out=wcombo, in_=wcombo_d[:, :, :])
                    fcombo = sb.tile([K, FC], bf16)
                    nc.sync.dma_start(out=fcombo, in_=fcombo_d[:, :])
                t0 += ct
            ec2 = fcombo[:, 0:1]
            ident = fcombo[:, 1 : 1 + K]            # I64 bf16

            expS1 = sb.tile([P, NT, K], fp8)    # token-major exp(S1), /16-shifted
            sh1 = sb.tile([P, 1], f32)
            nc.vector.memset(sh1, -2.772588722239781)   # -ln(16): keeps exp < 240 (fp8 max)
            # dummy exp: pulls the 1.3us LoadActFuncSet into the DMA head
            warm = sb.tile([P, 1], f32)
            nc.scalar.activation(warm, sh1, Exp)
            expS2 = sb.tile([K, NC2, W], bf16)          # agent-major exp(S2)

            # vaF holder; constant ec2 column written early
            vaF = sb.tile([K, E], bf16)
            nc.vector.tensor_copy(vaF[:, D:E], ec2)

            # ---- phase 1: S1 slabs + free-axis-paired S2 chunks ----
            # deep PSUM rings so matmul(n+1) never waits exp(n); scoped so the
            # y phase can reuse all 8 banks afterwards
            with (
                tc.tile_pool(name="s1p", bufs=3, space="PSUM") as s1p,  # 3 banks
                tc.tile_pool(name="s2p", bufs=2, space="PSUM") as s2p,  # 4 banks
                tc.tile_pool(name="pX", bufs=1, space="PSUM") as pX,    # 1 bank
            ):
                avx_ps = pX.tile([K, E], f32, tag="pX")
                nslab = len(SLABS)
                sstart = [sum(SLABS[:i]) for i in range(nslab)]

                def s1_slab(b):
                    t0, sl = sstart[b], SLABS[b]
                    ps = s1p.tile([P, sl, K], f32, tag="s1p")
                    for j in range(sl):
                        t = t0 + j
                        # DoubleRow: 2 fp8 weights/cell -> 256-contraction in one mm
                        nc.tensor.matmul(
                            ps[:, j, :], xT[:, :, ts(t, P)], wk,
                            start=True, stop=True, perf_mode=DR,
                        )
                    nc.scalar.activation(
                        expS1[:, t0 : t0 + sl, :], ps, Exp,
                        scale=float(D ** -0.5), bias=sh1,
                    )
                    for j in range(sl // 2):
                        u = t0 // 2 + j
                        nc.tensor.matmul(
                            avx_ps, expS1[:, 2 * u : 2 * u + 2, :],
                            xe[:, 2 * u : 2 * u + 2, :],
                            start=(u == 0), stop=(u == NT // 2 - 1), perf_mode=DR,
                        )

                def s2_pair(h):
                    # two 512-token chunks in two PSUM banks -> one exp, free 1024
                    p2 = s2p.tile([K, 2, W], f32, tag="s2p")
                    for g in range(2):
                        c = 2 * h + g
                        nc.tensor.matmul(
                            p2[:, g, :], wq, xT[:, :, ts(c, W)],
                            start=True, stop=True, perf_mode=DR,
                        )
                    nc.scalar.activation(
                        expS2[:, 2 * h : 2 * h + 2, :], p2, Exp,
                        scale=float(D ** -0.5), bias=sh1[0:K, :],
                    )

                for b in range(nslab):
                    s1_slab(b)
                # S2 logits only feed the y phase; running them after the S1
                # stream keeps Act free so expS1 (which gates avx/vaF) never
                # queues behind a 1us S2 exp. They overlap the vaF chain.
                for h in range(NC2 // 2):
                    s2_pair(h)

                # ---- vaF[:, :D] = (avx*ec2/s1 @ Wbig) ----
                rec1 = sb.tile([K, 1], f32)
                nc.vector.reciprocal(rec1, avx_ps[:, D:E])
                nc.vector.tensor_tensor(rec1, rec1, ec2, mybir.AluOpType.mult)
                avx_s = sb.tile([K, D], bf16)
                nc.vector.tensor_scalar_mul(avx_s, avx_ps[:, :D], rec1)
                avxT = sb.tile([P, DS, K], bf16)
                tp = s1p.tile([P, DS, K], bf16, tag="s1p")
                for s in range(DS):
                    nc.tensor.transpose(tp[:, s, :], avx_s[:, ts(s, P)], ident)
                nc.vector.tensor_copy(avxT, tp)    # bf16: DVE 2x mode
                vf_ps = s1p.tile([K, D], f32, tag="s1p")
                for s in range(DS):
                    nc.tensor.matmul(
                        vf_ps, avxT[:, s, :], wcombo[:, s, :],
                        start=(s == 0), stop=(s == DS - 1),
                    )
                nc.vector.tensor_copy(vaF[:, 0:D], vf_ps)

            # ---- y_ext[n, :] = sum_k expS2[k,n] * vaF_ext[k, :] ----
            # col D of vaF_ext is ec2, so col D of y_ext = s2. 4-tile groups in
            # a bufs=2 pool pipeline matmul vs cast; casts alternate Act/DVE
            # (both idle here); one DMA per group trims the last-DMA tail.
            with tc.tile_pool(name="ypool", bufs=2, space="PSUM") as ypool:
                g0 = 0
                for gi, gsz in enumerate(YGROUPS):
                    yp = ypool.tile([P, 4, W], f32, tag="ypool")
                    y_sb = yout.tile([P, gsz, E], fp8, tag="ysb")
                    for j in range(gsz):
                        t = g0 + j
                        nc.tensor.matmul(
                            yp[:, j, 0:E],
                            expS2[:, t // 4, ts(t % 4, P)],
                            vaF, start=True, stop=True,
                        )
                    if gi % 2 == 0:
                        nc.scalar.activation(y_sb, yp[:, 0:gsz, 0:E], Copy)
                    else:
                        nc.vector.tensor_copy(y_sb, yp[:, 0:gsz, 0:E])
                    nc.sync.dma_start(out=ye_d[:, g0 : g0 + gsz, :], in_=y_sb)
                    g0 += gsz

    nc.compile()
    return nc


def _get_nc():
    if "nc" not in _CACHE:
        _CACHE["nc"] = _build_nc()
    return _CACHE["nc"]


def _prepare_in_maps(agent, x, W_qkv, b_qkv, W_agent, b_agent, W_fc1, b_fc1, W_fc2, b_fc2):
    # ---- host folds (float64 for stability, cast down at the end) ----
    agent64 = np.asarray(agent, np.float64)
    Wqkv64 = np.asarray(W_qkv, np.float64)
    bqkv64 = np.asarray(b_qkv, np.float64)
    Wag64 = np.asarray(W_agent, np.float64)
    bag64 = np.asarray(b_agent, np.float64)
    Wf1 = np.asarray(W_fc1, np.float64)
    bf1 = np.asarray(b_fc1, np.float64)
    Wf2 = np.asarray(W_fc2, np.float64)
    bf2 = np.asarray(b_fc2, np.float64)

    ag = agent64 @ Wag64 + bag64
    q_agent, k_agent = ag[:, :D], ag[:, D:]
    W_q, W_k, W_v = Wqkv64[:, :D], Wqkv64[:, D : 2 * D], Wqkv64[:, 2 * D :]
    b_q, b_v = bqkv64[:D], bqkv64[2 * D :]

    wk_f = W_k @ q_agent.T                      # [D, K]
    wq_f = W_q @ k_agent.T                      # [D, K]
    c2_f = (D ** -0.5) * (k_agent @ b_q)        # [K]
    ec2_f = np.exp(c2_f)                        # [K]
    Wbig = W_v @ Wf1 @ Wf2                      # [D, D]
    bbig = (b_v @ Wf1 + bf1) @ Wf2 + bf2        # [D], added on host

    # [D, D] -> [P, DS, D] with d = s*128 + p
    wcombo_b = np.ascontiguousarray(
        Wbig.reshape(DS, P, D).transpose(1, 0, 2)
    ).astype(_BF16)
    wkq8 = np.concatenate([wk_f, wq_f], axis=1).reshape(DS, P, 2 * K)
    wkq8 = np.ascontiguousarray(wkq8.transpose(1, 0, 2)).astype(_FP8)
    fcombo = np.zeros((K, 1 + K), np.float32)
    fcombo[:, 0] = ec2_f
    fcombo[:, 1 :] = np.eye(K)
    fcombo = np.ascontiguousarray(fcombo).astype(_BF16)

    x32 = np.asarray(x, np.float32)
    # xe pack: [B, N, E] -> [B, P, NT, E], token = t*128 + p
    xb = np.ones((B, N, E), _FP8)
    xb[:, :, :D] = x32.astype(_FP8)
    xeb = np.ascontiguousarray(xb.reshape(B, NT, P, E).transpose(0, 2, 1, 3))
    # xT pack: [B, D, N] -> [B, P, DS, N], d = s*128 + p
    xTb = x32.transpose(0, 2, 1).reshape(B, DS, P, N)
    xTb = np.ascontiguousarray(xTb.transpose(0, 2, 1, 3)).astype(_FP8)

    in_maps = [
        {
            "xT": xTb[i],
            "xe": xeb[i],
            "wkq8": wkq8,
            "wcombo": wcombo_b,
            "fcombo": fcombo,
        }
        for i in range(B)
    ]

    return in_maps, x32, bbig.astype(np.float32)


def kernel(**inputs):
    from concourse.bass_utils import run_bass_kernel_spmd

    in_maps, x32, bbig = _prepare_in_maps(**inputs)
    nc = _get_nc()
    res_obj = run_bass_kernel_spmd(nc, in_maps, core_ids=list(range(B)))
    _CACHE["last_results"] = res_obj
    res = res_obj.results

    # ye [P, NT, E] -> [N, E] with token = t*128 + p
    ye = np.stack(
        [np.asarray(res[i]["ye"]).transpose(1, 0, 2).reshape(N, E) for i in range(B)]
    ).astype(np.float32)
    out = ye[:, :, :D] / ye[:, :, D:E] + bbig[None, None, :] + x32
    return out.astype(np.float32)


# revision 13
# speedup vs baseline: 1.3329x; 1.0500x over previous
"""Trainium2 Bass kernel for agent-attention (AAGA): 8-core data-parallel over batch.

Math (per batch b):
  qkv = x @ W_qkv + b_qkv ; q,k,v = split(qkv)
  ag  = agent @ W_agent + b_agent ; q_agent,k_agent = split(ag)
  attn1 = softmax(q_agent @ k^T * s)        # [K, N]
  va    = (attn1 @ v) @ W_fc1 + b_fc1       # [K, d]
  attn2 = softmax(q @ k_agent^T * s)        # [N, K]
  out   = (attn2 @ va) @ W_fc2 + b_fc2 + x  # [N, d]

Host-side algebraic folds (everything not involving x is an input):
  q_agent/k_agent computed on host; q,k,v never materialized on device.
  S1^T = x @ (W_k@q_agent^T): b_k drops out of the softmax (shift invariance).
  va-chain: attn1 rows sum to 1, so all later biases fold into a single
       constant row bbig = (b_v@W_fc1+b_fc1)@W_fc2 + b_fc2 ADDED ON HOST.
  Device vaF[k,:] = [ (ec2/s1)[k] * (expS1^T x)[k,:] @ Wbig | ec2[k] ],
  with Wbig = W_v@W_fc1@W_fc2. Then y = expS2^T @ vaF gives
  y[:, :D] = attn2-numerator combination and y[:, D] = s2 (denominator).
  Host epilogue: out = y[:, :D]/y[:, D] + bbig + x  (exact fp32).

DMA/engine regime (cost model): each DMACopy costs ~625ns on a single
serialized HWDGE queue (SP/Act) or ~1us on the Pool engine via SWDGE;
transfers serialize on DMA_ENGINES at 360GB/s with a 2x penalty for
runs <512B. GPSIMD cannot touch PSUM, so all PSUM->SBUF casts go on
DVE+Act, batched 3-4 token-tiles per instruction via multi-bank PSUM
tiles, alternating two PSUM pools so matmul and copy pipeline.
"""

import numpy as np
import ml_dtypes

B, N, D, K = 8, 4096, 256, 64
E = D + 1          # ones-column appended
P = 128
NT = N // P        # 32 token tiles
DS = D // P        # 2 contraction subtiles
W = 512            # free-dim chunk for S2^T
NC2 = N // W       # 8 chunks

# input streaming chunks (in token tiles); small first chunk starts compute
# early, small last chunk cuts the avx tail
XCHUNKS = [4, 8, 8, 8, 4]
# S1 slabs (in token tiles); small last slab shortens the avx tail
SLABS = [4, 8, 8, 8, 4]
# y-phase: 2-tile PSUM groups in a 4-deep ring (ring latency off the
# critical path); one output DMA per 4 tiles, alternating SP/Pool issue
YGROUPS = [2] * 16

_BF16 = ml_dtypes.bfloat16
_FP8 = ml_dtypes.float8_e4m3

_CACHE = {}


def _build_nc():
    import concourse.bass as bass
    import concourse.tile as tile
    from concourse import bacc, mybir

    f32 = mybir.dt.float32
    bf16 = mybir.dt.bfloat16
    fp8 = mybir.dt.float8e4
    Exp = mybir.ActivationFunctionType.Exp
    DR = mybir.MatmulPerfMode.DoubleRow
    Copy = mybir.ActivationFunctionType.Copy
    ts = bass.ts

    nc = bacc.Bacc("TRN2", target_bir_lowering=False, debug=False)

    xT_d = nc.declare_dram_parameter("xT", [P, DS, N], fp8, isOutput=False)
    xe_d = nc.declare_dram_parameter("xe", [P, NT, E], fp8, isOutput=False)
    wkq8_d = nc.declare_dram_parameter("wkq8", [P, DS, 2 * K], fp8, isOutput=False)
    wcombo_d = nc.declare_dram_parameter("wcombo", [P, DS, D], bf16, isOutput=False)
    FC = 1 + K         # [ec2 | I64]
    fcombo_d = nc.declare_dram_parameter("fcombo", [K, FC], bf16, isOutput=False)
    ye_d = nc.declare_dram_parameter("ye", [P, NT, E], fp8, isOutput=True)

    with tile.TileContext(nc) as tc:
        with (
            tc.tile_pool(name="sb", bufs=1) as sb,
            tc.tile_pool(name="yout", bufs=4) as yout,
        ):
            # ---------------- input DMAs ----------------
            wkq8 = sb.tile([P, DS, 2 * K], fp8)
            nc.sync.dma_start(out=wkq8, in_=wkq8_d[:, :, :])
            wk = wkq8[:, :, 0:K]
            wq = wkq8[:, :, K : 2 * K]

            xT = sb.tile([P, DS, N], fp8)
            xe = sb.tile([P, NT, E], fp8)
            t0 = 0
            for ci, ct in enumerate(XCHUNKS):
                nc.sync.dma_start(
                    out=xT[:, :, P * t0 : P * (t0 + ct)],
                    in_=xT_d[:, :, P * t0 : P * (t0 + ct)],
                )
                nc.gpsimd.dma_start(
                    out=xe[:, t0 : t0 + ct, :], in_=xe_d[:, t0 : t0 + ct, :]
                )
                if ci == 1:
                    wcombo = sb.tile([P, DS, D], bf16)
                    nc.sync.dma_start(out=wcombo, in_=wcombo_d[:, :, :])
                    fcombo = sb.tile([K, FC], bf16)
                    nc.sync.dma_start(out=fcombo, in_=fcombo_d[:, :])
                t0 += ct
            ec2 = fcombo[:, 0:1]
            ident = fcombo[:, 1 : 1 + K]            # I64 bf16

            expS1 = sb.tile([P, NT, K], fp8)    # token-major exp(S1), /16-shifted
            sh1 = sb.tile([P, 1], f32)
            nc.vector.memset(sh1, -2.772588722239781)   # -ln(16): keeps exp < 240 (fp8 max)
            # dummy exp: pulls the 1.3us LoadActFuncSet into the DMA head
            warm = sb.tile([P, 1], f32)
            nc.scalar.activation(warm, sh1, Exp)
            expS2 = sb.tile([K, NC2, W], bf16)          # agent-major exp(S2)

            # vaF holder; constant ec2 column written early
            vaF = sb.tile([K, E], bf16)
            nc.vector.tensor_copy(vaF[:, D:E], ec2)

            # ---- phase 1: S1 slabs + free-axis-paired S2 chunks ----
            # deep PSUM rings so matmul(n+1) never waits exp(n); scoped so the
            # y phase can reuse all 8 banks afterwards
            with (
                tc.tile_pool(name="s1p", bufs=3, space="PSUM") as s1p,  # 3 banks
                tc.tile_pool(name="s2p", bufs=2, space="PSUM") as s2p,  # 4 banks
                tc.tile_pool(name="pX", bufs=1, space="PSUM") as pX,    # 1 bank
            ):
                avx_ps = pX.tile([K, E], f32, tag="pX")
                nslab = len(SLABS)
                sstart = [sum(SLABS[:i]) for i in range(nslab)]

                def s1_slab(b):
                    t0, sl = sstart[b], SLABS[b]
                    ps = s1p.tile([P, sl, K], f32, tag="s1p")
                    for j in range(sl):
                        t = t0 + j
                        # DoubleRow: 2 fp8 weights/cell -> 256-contraction in one mm
                        nc.tensor.matmul(
                            ps[:, j, :], xT[:, :, ts(t, P)], wk,
                            start=True, stop=True, perf_mode=DR,
                        )
                    nc.scalar.activation(
                        expS1[:, t0 : t0 + sl, :], ps, Exp,
                        scale=float(D ** -0.5), bias=sh1,
                    )
                    for j in range(sl // 2):
                        u = t0 // 2 + j
                        nc.tensor.matmul(
                            avx_ps, expS1[:, 2 * u : 2 * u + 2, :],
                            xe[:, 2 * u : 2 * u + 2, :],
                            start=(u == 0), stop=(u == NT // 2 - 1), perf_mode=DR,
                        )

                def s2_pair(h):
                    # two 512-token chunks in two PSUM banks -> one exp, free 1024
                    p2 = s2p.tile([K, 2, W], f32, tag="s2p")
                    for g in range(2):
                        c = 2 * h + g
                        nc.tensor.matmul(
                            p2[:, g, :], wq, xT[:, :, ts(c, W)],
                            start=True, stop=True, perf_mode=DR,
                        )
                    nc.scalar.activation(
                        expS2[:, 2 * h : 2 * h + 2, :], p2, Exp,
                        scale=float(D ** -0.5), bias=sh1[0:K, :],
                    )

                for b in range(nslab):
                    s1_slab(b)
                # S2 logits only feed the y phase; running them after the S1
                # stream keeps Act free so expS1 (which gates avx/vaF) never
                # queues behind a 1us S2 exp. They overlap the vaF chain.
                for h in range(NC2 // 2):
                    s2_pair(h)

                # ---- vaF[:, :D] = (avx*ec2/s1 @ Wbig) ----
                rec1 = sb.tile([K, 1], f32)
                nc.vector.reciprocal(rec1, avx_ps[:, D:E])
                nc.vector.tensor_tensor(rec1, rec1, ec2, mybir.AluOpType.mult)
                avx_s = sb.tile([K, D], bf16)
                nc.vector.tensor_scalar_mul(avx_s, avx_ps[:, :D], rec1)
                avxT = sb.tile([P, DS, K], bf16)
                tp = s1p.tile([P, DS, K], bf16, tag="s1p")
                for s in range(DS):
                    nc.tensor.transpose(tp[:, s, :], avx_s[:, ts(s, P)], ident)
                nc.vector.tensor_copy(avxT, tp)    # bf16: DVE 2x mode
                vf_ps = s1p.tile([K, D], f32, tag="s1p")
                for s in range(DS):
                    nc.tensor.matmul(
                        vf_ps, avxT[:, s, :], wcombo[:, s, :],
                        start=(s == 0), stop=(s == DS - 1),
                    )
                nc.vector.tensor_copy(vaF[:, 0:D], vf_ps)

            # ---- y_ext[n, :] = sum_k expS2[k,n] * vaF_ext[k, :] ----
            # col D of vaF_ext is ec2, so col D of y_ext = s2. 2-tile groups in
            # a bufs=4 PSUM ring so the copy->matmul ring latency is amortized
            # 4-deep; casts alternate Act/DVE; DMA per 4 tiles, SP/Pool alternating.
            with tc.tile_pool(name="ypool", bufs=4, space="PSUM") as ypool:
                g0 = 0
                y_sb = None
                for gi, gsz in enumerate(YGROUPS):
                    yp = ypool.tile([P, 2, W], f32, tag="ypool")
                    if gi % 2 == 0:
                        ysz = gsz if gi == len(YGROUPS) - 1 else gsz + YGROUPS[gi + 1]
                        y_sb = yout.tile([P, ysz, E], fp8, tag="ysb")
                        ysb0 = g0
                    for j in range(gsz):
                        t = g0 + j
                        nc.tensor.matmul(
                            yp[:, j, 0:E],
                            expS2[:, t // 4, ts(t % 4, P)],
                            vaF, start=True, stop=True,
                        )
                    dst = y_sb[:, g0 - ysb0 : g0 - ysb0 + gsz, :]
                    if gi % 2 == 0:
                        nc.scalar.activation(dst, yp[:, 0:gsz, 0:E], Copy)
                    else:
                        nc.vector.tensor_copy(dst, yp[:, 0:gsz, 0:E])
                    if gi % 2 == 1 or gi == len(YGROUPS) - 1:
                        qn = g0 + gsz - ysb0
                        eng = nc.sync if (gi // 2) % 2 == 0 else nc.gpsimd
                        eng.dma_start(
                            out=ye_d[:, ysb0 : ysb0 + qn, :], in_=y_sb[:, 0:qn, :]
                        )
                    g0 += gsz

    nc.compile()
    return nc


def _get_nc():
    if "nc" not in _CACHE:
        _CACHE["nc"] = _build_nc()
    return _CACHE["nc"]


def _prepare_in_maps(agent, x, W_qkv, b_qkv, W_agent, b_agent, W_fc1, b_fc1, W_fc2, b_fc2):
    # ---- host folds (float64 for stability, cast down at the end) ----
    agent64 = np.asarray(agent, np.float64)
    Wqkv64 = np.asarray(W_qkv, np.float64)
    bqkv64 = np.asarray(b_qkv, np.float64)
    Wag64 = np.asarray(W_agent, np.float64)
    bag64 = np.asarray(b_agent, np.float64)
    Wf1 = np.asarray(W_fc1, np.float64)
    bf1 = np.asarray(b_fc1, np.float64)
    Wf2 = np.asarray(W_fc2, np.float64)
    bf2 = np.asarray(b_fc2, np.float64)

    ag = agent64 @ Wag64 + bag64
    q_agent, k_agent = ag[:, :D], ag[:, D:]
    W_q, W_k, W_v = Wqkv64[:, :D], Wqkv64[:, D : 2 * D], Wqkv64[:, 2 * D :]
    b_q, b_v = bqkv64[:D], bqkv64[2 * D :]

    wk_f = W_k @ q_agent.T                      # [D, K]
    wq_f = W_q @ k_agent.T                      # [D, K]
    c2_f = (D ** -0.5) * (k_agent @ b_q)        # [K]
    ec2_f = np.exp(c2_f)                        # [K]
    Wbig = W_v @ Wf1 @ Wf2                      # [D, D]
    bbig = (b_v @ Wf1 + bf1) @ Wf2 + bf2        # [D], added on host

    # [D, D] -> [P, DS, D] with d = s*128 + p
    wcombo_b = np.ascontiguousarray(
        Wbig.reshape(DS, P, D).transpose(1, 0, 2)
    ).astype(_BF16)
    wkq8 = np.concatenate([wk_f, wq_f], axis=1).reshape(DS, P, 2 * K)
    wkq8 = np.ascontiguousarray(wkq8.transpose(1, 0, 2)).astype(_FP8)
    fcombo = np.zeros((K, 1 + K), np.float32)
    fcombo[:, 0] = ec2_f
    fcombo[:, 1 :] = np.eye(K)
    fcombo = np.ascontiguousarray(fcombo).astype(_BF16)

    x32 = np.asarray(x, np.float32)
    # xe pack: [B, N, E] -> [B, P, NT, E], token = t*128 + p
    xb = np.ones((B, N, E), _FP8)
    xb[:, :, :D] = x32.astype(_FP8)
    xeb = np.ascontiguousarray(xb.reshape(B, NT, P, E).transpose(0, 2, 1, 3))
    # xT pack: [B, D, N] -> [B, P, DS, N], d = s*128 + p
    xTb = x32.transpose(0, 2, 1).reshape(B, DS, P, N)
    xTb = np.ascontiguousarray(xTb.transpose(0, 2, 1, 3)).astype(_FP8)

    in_maps = [
        {
            "xT": xTb[i],
            "xe": xeb[i],
            "wkq8": wkq8,
            "wcombo": wcombo_b,
            "fcombo": fcombo,
        }
        for i in range(B)
    ]

    return in_maps, x32, bbig.astype(np.float32)


def kernel(**inputs):
    from concourse.bass_utils import run_bass_kernel_spmd

    in_maps, x32, bbig = _prepare_in_maps(**inputs)
    nc = _get_nc()
    res_obj = run_bass_kernel_spmd(nc, in_maps, core_ids=list(range(B)))
    _CACHE["last_results"] = res_obj
    res = res_obj.results

    # ye [P, NT, E] -> [N, E] with token = t*128 + p
    ye = np.stack(
        [np.asarray(res[i]["ye"]).transpose(1, 0, 2).reshape(N, E) for i in range(B)]
    ).astype(np.float32)
    out = ye[:, :, :D] / ye[:, :, D:E] + bbig[None, None, :] + x32
    return out.astype(np.float32)


# revision 15
# speedup vs baseline: 1.3966x; 1.0478x over previous
"""Trainium2 Bass kernel for agent-attention (AAGA): 8-core data-parallel over batch.

Math (per batch b):
  qkv = x @ W_qkv + b_qkv ; q,k,v = split(qkv)
  ag  = agent @ W_agent + b_agent ; q_agent,k_agent = split(ag)
  attn1 = softmax(q_agent @ k^T * s)        # [K, N]
  va    = (attn1 @ v) @ W_fc1 + b_fc1       # [K, d]
  attn2 = softmax(q @ k_agent^T * s)        # [N, K]
  out   = (attn2 @ va) @ W_fc2 + b_fc2 + x  # [N, d]

Host-side algebraic folds (everything not involving x is an input):
  q_agent/k_agent computed on host; q,k,v never materialized on device.
  S1^T = x @ (W_k@q_agent^T): b_k drops out of the softmax (shift invariance).
  va-chain: attn1 rows sum to 1, so all later biases fold into a single
       constant row bbig = (b_v@W_fc1+b_fc1)@W_fc2 + b_fc2 ADDED ON HOST.
  Device vaF[k,:] = [ (ec2/s1)[k] * (expS1^T x)[k,:] @ Wbig | ec2[k] ],
  with Wbig = W_v@W_fc1@W_fc2. Then y = expS2^T @ vaF gives
  y[:, :D] = attn2-numerator combination and y[:, D] = s2 (denominator).
  Host epilogue: out = y[:, :D]/y[:, D] + bbig + x  (exact fp32).

DMA/engine regime (cost model): each DMACopy costs ~625ns on a single
serialized HWDGE queue (SP/Act) or ~1us on the Pool engine via SWDGE;
transfers serialize on DMA_ENGINES at 360GB/s with a 2x penalty for
runs <512B. GPSIMD cannot touch PSUM, so all PSUM->SBUF casts go on
DVE+Act, batched 3-4 token-tiles per instruction via multi-bank PSUM
tiles, alternating two PSUM pools so matmul and copy pipeline.
"""

import numpy as np
import ml_dtypes

B, N, D, K = 8, 4096, 256, 64
E = D + 1          # ones-column appended
P = 128
NT = N // P        # 32 token tiles
DS = D // P        # 2 contraction subtiles
W = 512            # free-dim chunk for S2^T
NC2 = N // W       # 8 chunks

# input streaming chunks (in token tiles); small first chunk starts compute
# early, small last chunk cuts the avx tail
XCHUNKS = [4, 8, 8, 8, 4]
# S1 slabs (in token tiles); small last slab shortens the avx tail
SLABS = [4, 8, 8, 8, 4]
# y-phase: 2-tile PSUM groups in a 4-deep ring (ring latency off the
# critical path); one output DMA per 4 tiles, alternating SP/Pool issue
YGROUPS = [2] * 16

_BF16 = ml_dtypes.bfloat16
_FP8 = ml_dtypes.float8_e4m3

_CACHE = {}


def _build_nc():
    import concourse.bass as bass
    import concourse.tile as tile
    from concourse import bacc, mybir

    f32 = mybir.dt.float32
    bf16 = mybir.dt.bfloat16
    fp8 = mybir.dt.float8e4
    Exp = mybir.ActivationFunctionType.Exp
    DR = mybir.MatmulPerfMode.DoubleRow
    Copy = mybir.ActivationFunctionType.Copy
    ts = bass.ts

    nc = bacc.Bacc("TRN2", target_bir_lowering=False, debug=False)

    xT_d = nc.declare_dram_parameter("xT", [P, DS, N], fp8, isOutput=False)
    xe_d = nc.declare_dram_parameter("xe", [P, NT, E], fp8, isOutput=False)
    wkq8_d = nc.declare_dram_parameter("wkq8", [P, DS, 2 * K], fp8, isOutput=False)
    wcombo_d = nc.declare_dram_parameter("wcombo", [P, DS, D], bf16, isOutput=False)
    FC = 1 + K         # [ec2 | I64]
    fcombo_d = nc.declare_dram_parameter("fcombo", [K, FC], bf16, isOutput=False)
    ye_d = nc.declare_dram_parameter("ye", [P, NT, E], fp8, isOutput=True)

    with tile.TileContext(nc) as tc:
        with (
            tc.tile_pool(name="sb", bufs=1) as sb,
            tc.tile_pool(name="yout", bufs=6) as yout,
        ):
            # ---------------- input DMAs ----------------
            wkq8 = sb.tile([P, DS, 2 * K], fp8)
            nc.sync.dma_start(out=wkq8, in_=wkq8_d[:, :, :])
            wk = wkq8[:, :, 0:K]
            wq = wkq8[:, :, K : 2 * K]

            xT = sb.tile([P, DS, N], fp8)
            xe = sb.tile([P, NT, E], fp8)
            t0 = 0
            for ci, ct in enumerate(XCHUNKS):
                nc.sync.dma_start(
                    out=xT[:, :, P * t0 : P * (t0 + ct)],
                    in_=xT_d[:, :, P * t0 : P * (t0 + ct)],
                )
                nc.gpsimd.dma_start(
                    out=xe[:, t0 : t0 + ct, :], in_=xe_d[:, t0 : t0 + ct, :]
                )
                if ci == 1:
                    wcombo = sb.tile([P, DS, D], bf16)
                    nc.sync.dma_start(out=wcombo, in_=wcombo_d[:, :, :])
                    fcombo = sb.tile([K, FC], bf16)
                    nc.sync.dma_start(out=fcombo, in_=fcombo_d[:, :])
                t0 += ct
            bias2 = fcombo[:, 0:1]                  # c2*scale - ln(16)
            ident = fcombo[:, 1 : 1 + K]            # I64 bf16

            expS1 = sb.tile([P, NT, K], fp8)    # token-major exp(S1), /16-shifted
            sh1 = sb.tile([P, 1], f32)
            nc.vector.memset(sh1, -2.772588722239781)   # -ln(16): keeps exp < 240 (fp8 max)
            # dummy exp: pulls the 1.3us LoadActFuncSet into the DMA head
            warm = sb.tile([P, 1], f32)
            nc.scalar.activation(warm, sh1, Exp)
            expS2 = sb.tile([K, NC2, W], bf16)          # agent-major exp(S2)

            # vaF holder; col D = 1.0 so y col D = s2 (c2 folded into bias2)
            vaF = sb.tile([K, E], bf16)
            nc.vector.memset(vaF[:, D:E], 1.0)

            # ---- phase 1: S1 slabs + free-axis-paired S2 chunks ----
            # deep PSUM rings so matmul(n+1) never waits exp(n); scoped so the
            # y phase can reuse all 8 banks afterwards
            with (
                tc.tile_pool(name="s1p", bufs=3, space="PSUM") as s1p,  # 3 banks
                tc.tile_pool(name="s2p", bufs=2, space="PSUM") as s2p,  # 4 banks
                tc.tile_pool(name="pX", bufs=1, space="PSUM") as pX,    # 1 bank
            ):
                avx_ps = pX.tile([K, E], f32, tag="pX")
                nslab = len(SLABS)
                sstart = [sum(SLABS[:i]) for i in range(nslab)]

                def s1_slab(b):
                    t0, sl = sstart[b], SLABS[b]
                    ps = s1p.tile([P, sl, K], f32, tag="s1p")
                    for j in range(sl):
                        t = t0 + j
                        # DoubleRow: 2 fp8 weights/cell -> 256-contraction in one mm
                        nc.tensor.matmul(
                            ps[:, j, :], xT[:, :, ts(t, P)], wk,
                            start=True, stop=True, perf_mode=DR,
                        )
                    nc.scalar.activation(
                        expS1[:, t0 : t0 + sl, :], ps, Exp,
                        scale=float(D ** -0.5), bias=sh1,
                    )
                    for j in range(sl // 2):
                        u = t0 // 2 + j
                        nc.tensor.matmul(
                            avx_ps, expS1[:, 2 * u : 2 * u + 2, :],
                            xe[:, 2 * u : 2 * u + 2, :],
                            start=(u == 0), stop=(u == NT // 2 - 1), perf_mode=DR,
                        )

                def s2_pair(h):
                    # two 512-token chunks in two PSUM banks -> one exp, free 1024
                    p2 = s2p.tile([K, 2, W], f32, tag="s2p")
                    for g in range(2):
                        c = 2 * h + g
                        nc.tensor.matmul(
                            p2[:, g, :], wq, xT[:, :, ts(c, W)],
                            start=True, stop=True, perf_mode=DR,
                        )
                    nc.scalar.activation(
                        expS2[:, 2 * h : 2 * h + 2, :], p2, Exp,
                        scale=float(D ** -0.5), bias=bias2,
                    )

                for b in range(nslab):
                    s1_slab(b)
                # S2 logits only feed the y phase; running them after the S1
                # stream keeps Act free so expS1 (which gates avx/vaF) never
                # queues behind a 1us S2 exp. They overlap the vaF chain.
                for h in range(NC2 // 2):
                    s2_pair(h)

                # ---- vaF[:, :D] = (avx/s1 @ Wbig); c2 lives in the exp bias ----
                rec1 = sb.tile([K, 1], f32)
                nc.vector.reciprocal(rec1, avx_ps[:, D:E])
                avx_s = sb.tile([K, D], bf16)
                avxT = sb.tile([P, DS, K], bf16)
                tp = s1p.tile([P, DS, K], bf16, tag="s1p")
                for s in range(DS):
                    # per-half scale then transpose: transpose s starts as soon
                    # as its half of avx_s is written
                    nc.vector.tensor_scalar_mul(
                        avx_s[:, ts(s, P)], avx_ps[:, ts(s, P)], rec1
                    )
                    nc.tensor.transpose(tp[:, s, :], avx_s[:, ts(s, P)], ident)
                nc.vector.tensor_copy(avxT, tp)    # bf16: DVE 2x mode
                vf_ps = s1p.tile([K, D], f32, tag="s1p")
                for s in range(DS):
                    nc.tensor.matmul(
                        vf_ps, avxT[:, s, :], wcombo[:, s, :],
                        start=(s == 0), stop=(s == DS - 1),
                    )
                nc.vector.tensor_copy(vaF[:, 0:D], vf_ps)

            # ---- y_ext[n, :] = sum_k expS2[k,n] * vaF_ext[k, :] ----
            # col D of vaF_ext is ec2, so col D of y_ext = s2. 2-tile groups in
            # a bufs=4 PSUM ring so the copy->matmul ring latency is amortized
            # 4-deep; casts alternate Act/DVE; DMA per 4 tiles, SP/Pool alternating.
            with tc.tile_pool(name="ypool", bufs=4, space="PSUM") as ypool:
                g0 = 0
                y_sb = None
                for gi, gsz in enumerate(YGROUPS):
                    yp = ypool.tile([P, 2, W], f32, tag="ypool")
                    solo = gi >= len(YGROUPS) - 2
                    if gi % 2 == 0 or solo:
                        ysz = gsz if solo else gsz + YGROUPS[gi + 1]
                        y_sb = yout.tile([P, ysz, E], fp8, tag="ysb")
                        ysb0 = g0
                    for j in range(gsz):
                        t = g0 + j
                        nc.tensor.matmul(
                            yp[:, j, 0:E],
                            expS2[:, t // 4, ts(t % 4, P)],
                            vaF, start=True, stop=True,
                        )
                    dst = y_sb[:, g0 - ysb0 : g0 - ysb0 + gsz, :]
                    if gi % 2 == 0:
                        nc.scalar.activation(dst, yp[:, 0:gsz, 0:E], Copy)
                    else:
                        nc.vector.tensor_copy(dst, yp[:, 0:gsz, 0:E])
                    if gi % 2 == 1 or solo:
                        qn = g0 + gsz - ysb0
                        eng = nc.gpsimd if (gi // 2) % 2 == 1 and not solo else nc.sync
                        eng.dma_start(
                            out=ye_d[:, ysb0 : ysb0 + qn, :], in_=y_sb[:, 0:qn, :]
                        )
                    g0 += gsz

    nc.compile()
    return nc


def _get_nc():
    if "nc" not in _CACHE:
        _CACHE["nc"] = _build_nc()
    return _CACHE["nc"]


def _prepare_in_maps(agent, x, W_qkv, b_qkv, W_agent, b_agent, W_fc1, b_fc1, W_fc2, b_fc2):
    # ---- host folds (float64 for stability, cast down at the end) ----
    agent64 = np.asarray(agent, np.float64)
    Wqkv64 = np.asarray(W_qkv, np.float64)
    bqkv64 = np.asarray(b_qkv, np.float64)
    Wag64 = np.asarray(W_agent, np.float64)
    bag64 = np.asarray(b_agent, np.float64)
    Wf1 = np.asarray(W_fc1, np.float64)
    bf1 = np.asarray(b_fc1, np.float64)
    Wf2 = np.asarray(W_fc2, np.float64)
    bf2 = np.asarray(b_fc2, np.float64)

    ag = agent64 @ Wag64 + bag64
    q_agent, k_agent = ag[:, :D], ag[:, D:]
    W_q, W_k, W_v = Wqkv64[:, :D], Wqkv64[:, D : 2 * D], Wqkv64[:, 2 * D :]
    b_q, b_v = bqkv64[:D], bqkv64[2 * D :]

    wk_f = W_k @ q_agent.T                      # [D, K]
    wq_f = W_q @ k_agent.T                      # [D, K]
    c2_f = (D ** -0.5) * (k_agent @ b_q)        # [K]
    ec2_f = np.exp(c2_f)                        # [K]
    Wbig = W_v @ Wf1 @ Wf2                      # [D, D]
    bbig = (b_v @ Wf1 + bf1) @ Wf2 + bf2        # [D], added on host

    # [D, D] -> [P, DS, D] with d = s*128 + p
    wcombo_b = np.ascontiguousarray(
        Wbig.reshape(DS, P, D).transpose(1, 0, 2)
    ).astype(_BF16)
    wkq8 = np.concatenate([wk_f, wq_f], axis=1).reshape(DS, P, 2 * K)
    wkq8 = np.ascontiguousarray(wkq8.transpose(1, 0, 2)).astype(_FP8)
    fcombo = np.zeros((K, 1 + K), np.float32)
    fcombo[:, 0] = c2_f - 2.772588722239781
    fcombo[:, 1 :] = np.eye(K)
    fcombo = np.ascontiguousarray(fcombo).astype(_BF16)

    x32 = np.asarray(x, np.float32)
    # xe pack: [B, N, E] -> [B, P, NT, E], token = t*128 + p
    xb = np.ones((B, N, E), _FP8)
    xb[:, :, :D] = x32.astype(_FP8)
    xeb = np.ascontiguousarray(xb.reshape(B, NT, P, E).transpose(0, 2, 1, 3))
    # xT pack: [B, D, N] -> [B, P, DS, N], d = s*128 + p
    xTb = x32.transpose(0, 2, 1).reshape(B, DS, P, N)
    xTb = np.ascontiguousarray(xTb.transpose(0, 2, 1, 3)).astype(_FP8)

    in_maps = [
        {
            "xT": xTb[i],
            "xe": xeb[i],
            "wkq8": wkq8,
            "wcombo": wcombo_b,
            "fcombo": fcombo,
        }
        for i in range(B)
    ]

    return in_maps, x32, bbig.astype(np.float32)


def kernel(**inputs):
    from concourse.bass_utils import run_bass_kernel_spmd

    in_maps, x32, bbig = _prepare_in_maps(**inputs)
    nc = _get_nc()
    res_obj = run_bass_kernel_spmd(nc, in_maps, core_ids=list(range(B)))
    _CACHE["last_results"] = res_obj
    res = res_obj.results

    # ye [P, NT, E] -> [N, E] with token = t*128 + p
    ye = np.stack(
        [np.asarray(res[i]["ye"]).transpose(1, 0, 2).reshape(N, E) for i in range(B)]
    ).astype(np.float32)
    out = ye[:, :, :D] / ye[:, :, D:E] + bbig[None, None, :] + x32
    return out.astype(np.float32)
